# revision 32
# baseline (speedup 1.0000x reference)
"""Trainium2 Bass kernel for nn_CategoryTheoryEngine (gnn_message_passing).

reference(...) returns only (combined_out [1,128], avg_tension scalar).

Dead-code analysis of the reference:
  - combined_out = softmax(tension) . out  -> needs per-cell out/tension only
  - avg_tension = mean(tension) + 0.1 * cat_tension
  - cat_tension depends on the GRU'd hidden state at the 64 morph_src/tgt
    rows only (limit/colimit are computed BEFORE the +0.05*diff update).
  - faction sync / debate / hid update only affect the discarded 3rd output.

So the device computes, data-parallel over cells (8 cores x 8192 cells):
  per cell: out = MLP_a(h) - MLP_g(h)  (x-part of layer 1 folded into bias),
  sumsq_c = sum_p out^2, w_c = exp(sumsq_c/128), and reduces
  V = sum_c w_c * out_c  [128],  SQ_p = sum_c out^2[p,c]  [128],  E = sum_c w_c.
Host combines the 8 partial results (combined = V/E) and computes the 64
morph rows' GRU -> limit/colimit -> cat_tension in numpy (0.1% of cells).
"""

import os
import sys
import types
from contextlib import ExitStack

import numpy as np

import concourse.bass as bass
import concourse.tile as tile
from concourse import mybir
from concourse.bass_utils import run_bass_kernel_spmd


def _ensure_ntff_hook():
    """The agent image's antenv lacks axon_hooks; recreate it so
    run_bass_kernel_spmd(trace=True) can NTFF-profile through axon."""
    try:
        from antenv.axon_hooks import get_axon_ntff_profile_hook  # noqa: F401

        return True
    except ImportError:
        pass
    try:
        import antenv

        if "/root/.axon_site" not in sys.path:
            sys.path.insert(0, "/root/.axon_site")
        from trn_agent_boot.trn_boot import _ntff_profile_via_ctypes

        mod = types.ModuleType("antenv.axon_hooks")
        state = {"hook": None}
        mod.set_axon_ntff_profile_hook = lambda h: state.__setitem__("hook", h)
        mod.get_axon_ntff_profile_hook = lambda: state["hook"]
        sys.modules["antenv.axon_hooks"] = mod
        antenv.axon_hooks = mod
        mod.set_axon_ntff_profile_hook(
            _ntff_profile_via_ctypes("/opt/axon/libaxon_pjrt.so"))
        return True
    except Exception as e:  # profiling is best-effort only
        print(f"[kernel] ntff hook install failed: {e}")
        return False

def _patch_tail_drain():
    """The stock kernel-tail emits ONE SP Drain waiting every proc's final
    semaphore tick; with 3 engines + several DMA queues that exceeds the
    Drain instruction's sync-wait slots and walrus refuses to codegen.
    Split the waits across several Drain instructions (<=4 waits each)."""
    if getattr(tile.TileContext, "_cte_split_drain", False):
        return
    from concourse.vector_clock import ScopedClock, VectorClock

    def _drain_and_barrier(self, tick_clock, wait_clock):
        gc = tick_clock.global_clock
        n = len(gc)
        procs = [i for i in range(n) if gc[i] > 0]
        groups = [[p] for p in procs] or [[]]
        for grp in groups:
            vec = [gc[i] if i in grp else 0 for i in range(n)]
            drain_inst = self.nc.sync.drain()
            wait_clock.add_sem_waits(
                drain_inst.ins, ScopedClock({None: VectorClock(vec)}))
        self.nc.all_engine_barrier()
        assert self.sems is not None
        popped = self.nc._tile_sem_poison_stack.pop()
        assert popped is self._sem_poison
        self.nc.clear_and_free_semaphores(
            list(self.sems.allocated().values()))
        self.nc.all_engine_barrier()

    tile.TileContext._drain_and_barrier = _drain_and_barrier
    tile.TileContext._cte_split_drain = True


_patch_tail_drain()

N_CELLS, IN_DIM, HID, OUT_DIM, MLP_H, N_MORPH = 65536, 128, 256, 128, 128, 32
N_CORES = 8
SHARD = N_CELLS // N_CORES  # 8192
NT = 512                    # cells per on-chip tile
NTILES = SHARD // NT        # 16
KC = HID // 128             # 2 contraction chunks for layer 1

F32 = mybir.dt.float32
F32R = mybir.dt.float32r
BF16 = mybir.dt.bfloat16

# 'f32r': fp32 storage, full-rate fp32r matmuls.  'bf16': bf16 storage+matmuls
# (halves DMA).  'f32': plain fp32 matmuls (4x slower, debug only).
MM_MODE = os.environ.get("CTE_MM_MODE", "f32r")

LAST_PROFILE = {}

_nc_cache = {}


def _data_dt():
    if MM_MODE == "bf16":
        return BF16
    if MM_MODE == "f32r":
        return F32R
    return F32


def _np_data_dt():
    if MM_MODE == "bf16":
        import ml_dtypes

        return ml_dtypes.bfloat16
    return np.float32


def _mm(ap):
    return ap


def _build_nc():
    if MM_MODE in _nc_cache:
        return _nc_cache[MM_MODE]

    dt = _data_dt()
    AF = mybir.ActivationFunctionType
    ALU = mybir.AluOpType

    nc = bass.Bass()
    # tiles 1..NTILES-1 of the hiddens shard (tile 0 rides in the const blob)
    hT = nc.declare_dram_parameter("hT", [128, KC, SHARD - NT], dt, False)
    # one const blob -> ONE DMA -> one semaphore for everything the first
    # matmul needs (PE matmul tolerates only a single sync wait):
    # free-dim layout: [0:768] weights (A1T k0,k1 | G1T k0,k1 | Wa2T | -Wg2T),
    # [768:771] bias cols (f32 bits: ba1_eff, bg1_eff, db), [771:772] pad,
    # [772:1796] tile-0 h_t (2 chunks x NT)
    CB_W, CB_B, CB_H0, CB_TOT = 0, 768, 772, 772 + KC * NT
    blob = nc.declare_dram_parameter("blob", [128, CB_TOT], dt, False)
    res = nc.declare_dram_parameter("res", [128, 4], F32, True)

    with tile.TileContext(nc) as tc, ExitStack() as ctx:
        consts = ctx.enter_context(tc.tile_pool(name="consts", bufs=1))
        loads = ctx.enter_context(tc.tile_pool(name="loads", bufs=1))
        work = ctx.enter_context(tc.tile_pool(name="work", bufs=4))
        accs = ctx.enter_context(tc.tile_pool(name="accs", bufs=1))
        wsb = ctx.enter_context(tc.tile_pool(name="wsb", bufs=NTILES))
        outs = ctx.enter_context(tc.tile_pool(name="outs", bufs=NTILES))
        # PSUM bank budget (8 banks): p_l1 x1, tp x1, p2 x2, wb x4.
        # wb needs depth 4 so its slot is provably free at schedule time --
        # its reader is the DVE product whose completion PE never observes,
        # and a PE matmul can carry only one sync wait.
        ps1 = ctx.enter_context(tc.tile_pool(name="ps1", bufs=1, space="PSUM"))
        ps2 = ctx.enter_context(tc.tile_pool(name="ps2", bufs=2, space="PSUM"))
        ps4 = ctx.enter_context(tc.tile_pool(name="ps4", bufs=4, space="PSUM"))

        blob_sb = consts.tile([128, CB_TOT], dt)
        nc.gpsimd.dma_start(out=blob_sb[:], in_=blob[:])
        wA1_sb = [blob_sb[:, k * MLP_H:(k + 1) * MLP_H] for k in range(KC)]
        wG1_sb = [blob_sb[:, (KC + k) * MLP_H:(KC + k + 1) * MLP_H]
                  for k in range(KC)]
        wA2_sb = blob_sb[:, 4 * MLP_H:5 * MLP_H]
        wG2n_sb = blob_sb[:, 5 * MLP_H:6 * MLP_H]
        bias_sb = blob_sb[:, CB_B:CB_B + 3].bitcast(F32)
        h0_sb = [blob_sb[:, CB_H0 + k * NT:CB_H0 + (k + 1) * NT]
                 for k in range(KC)]

        # ones produced on ACT (DVE memset cannot write float32r, and ACT
        # keeps the consumers' deps ACT-local): copy(x*0 + 1) = 1
        ones_col = consts.tile([128, 1], dt)
        nc.scalar.activation(out=ones_col[:], in_=bias_sb[:, 0:1],
                             func=AF.Copy, scale=0.0, bias=1.0)
        ones_row = consts.tile([1, 128], dt)
        nc.scalar.activation(out=ones_row[:], in_=blob_sb[0:1, 0:MLP_H],
                             func=AF.Copy, scale=0.0, bias=1.0)
        # zero produced on ACT (not DVE) so the exp's bias dep stays ACT-local;
        # doubles as the ACT pre-touch of the const-blob DMA
        zero1 = consts.tile([1, 1], F32)
        nc.scalar.mul(zero1[:], bias_sb[0:1, 0:1], 0.0)

        # scratch target for DVE guard copies (absorbs PE waits so the
        # product TensorTensor carries only its ACT wait)
        dve_gw = consts.tile([1, 1], F32)
        dve_gw2 = consts.tile([1, 1], F32)
        vacc = accs.tile([128, NTILES], F32)
        sqacc = accs.tile([128, NTILES], F32)
        eacc = accs.tile([1, NTILES], F32)

        # the 15 remaining tiles arrive in a few chunked DMAs (pipelining
        # vs. startup bubble; consumers slice the big resident tiles)
        NCHUNK = 5
        per = 3 * NT  # 3 tiles per chunk
        h_chunks = []
        for h in range(NCHUNK):
            lo = h * per
            hi = min(SHARD - NT, lo + per)
            ht_big = loads.tile([128, KC, per], dt, tag=f"ht_big{h}")
            nc.gpsimd.dma_start(out=ht_big[:, :, :hi - lo], in_=hT[:, :, lo:hi])
            h_chunks.append(ht_big)

        prev_prod = None
        for i in range(NTILES):
            if i == 0:
                h_k = h0_sb
            else:
                h, off = (i - 1) // 3, ((i - 1) % 3) * NT
                h_k = [h_chunks[h][:, k, off:off + NT] for k in range(KC)]

            # layer 1: a and g sequentially through one single-bank tag
            p1a = ps1.tile([128, NT], F32, tag="p_l1")
            for k in range(KC):
                nc.tensor.matmul(
                    p1a[:], _mm(wA1_sb[k]), _mm(h_k[k]),
                    start=(k == 0), stop=(k == KC - 1),
                )
            acts_a = work.tile([128, NT], dt)
            nc.scalar.activation(
                out=acts_a[:], in_=p1a[:], func=AF.Relu,
                bias=bias_sb[:, 0:1], scale=1.0,
            )
            p1g = ps1.tile([128, NT], F32, tag="p_l1")
            for k in range(KC):
                nc.tensor.matmul(
                    p1g[:], _mm(wG1_sb[k]), _mm(h_k[k]),
                    start=(k == 0), stop=(k == KC - 1),
                )
            acts_g = work.tile([128, NT], dt)
            nc.scalar.activation(
                out=acts_g[:], in_=p1g[:], func=AF.Relu,
                bias=bias_sb[:, 1:2], scale=1.0,
            )

            # layer 2: p2 = a' @ Wa2.T - g' @ Wg2.T  (= out - db)
            p2 = ps2.tile([128, NT], F32)
            nc.tensor.matmul(p2[:], _mm(wA2_sb), _mm(acts_a[:]),
                             start=True, stop=False)
            nc.tensor.matmul(p2[:], _mm(wG2n_sb), _mm(acts_g[:]),
                             start=False, stop=True)

            # out = p2 + db, materialized in SBUF (engines may read at most
            # one PSUM operand per instruction)
            out_sb = outs.tile([128, NT], F32)
            nc.scalar.activation(
                out=out_sb[:], in_=p2[:], func=AF.Identity,
                bias=bias_sb[:, 2:3], scale=1.0,
            )

            # sq = out^2 ; ACT accumulator gives sum over cells per partition
            sq = work.tile([128, NT], dt)
            nc.scalar.activation(
                out=sq[:], in_=out_sb[:], func=AF.Square,
                accum_out=sqacc[:, i : i + 1],
            )

            # tension*128 per cell: sum over partitions via ones-matmul
            tp = ps1.tile([1, NT], F32)
            nc.tensor.matmul(tp[:], _mm(ones_col[:]), _mm(sq[:]),
                             start=True, stop=True)

            # w = exp(t) ; accumulate sum of w
            w_sb = wsb.tile([1, NT], dt)
            nc.scalar.activation(
                out=w_sb[:], in_=tp[:], func=AF.Exp,
                bias=zero1[0:1, 0:1], scale=1.0 / OUT_DIM,
                accum_out=eacc[:, i : i + 1],
            )

            # broadcast w across partitions (rank-1 matmul). The ldweights
            # guard reads the previous tile's DVE product so PE observes the
            # DVE tick that releases this wb slot -- the real matmul then
            # carries only its ACT wait (PE matmuls support a single sync
            # wait). The stray weight load is overwritten by the matmul's
            # own self-loading LDWEIGHTS.
            wb = ps4.tile([128, NT], F32)
            if prev_prod is not None:
                nc.tensor.ldweights(prev_prod[:, 0:32].bitcast(BF16))
            nc.tensor.matmul(wb[:], _mm(ones_row[:]), _mm(w_sb[:]),
                             start=True, stop=True)

            # V partial: sum_c w_c * out[:, c]
            prod = work.tile([128, NT], F32)
            nc.vector.tensor_copy(out=dve_gw[0:1, 0:1], in_=wb[0:1, 0:1])
            nc.vector.tensor_copy(out=dve_gw2[0:1, 0:1], in_=out_sb[0:1, 0:1])
            nc.vector.tensor_mul(out=prod[:], in0=out_sb[:], in1=wb[:])
            nc.vector.tensor_reduce(
                out=vacc[:, i : i + 1], in_=prod[:],
                axis=mybir.AxisListType.X, op=ALU.add,
            )
            prev_prod = prod

        outsb = consts.tile([128, 4], F32)
        nc.vector.memset(outsb[:], 0.0)
        nc.vector.tensor_reduce(out=outsb[:, 0:1], in_=vacc[:],
                                axis=mybir.AxisListType.X, op=mybir.AluOpType.add)
        nc.vector.tensor_reduce(out=outsb[:, 1:2], in_=sqacc[:],
                                axis=mybir.AxisListType.X, op=mybir.AluOpType.add)
        nc.vector.tensor_reduce(out=outsb[0:1, 2:3], in_=eacc[:],
                                axis=mybir.AxisListType.X, op=mybir.AluOpType.add)
        nc.sync.dma_start(out=res[:], in_=outsb[:])

    nc.finalize()
    _nc_cache[MM_MODE] = nc
    return nc


def _sigmoid(v):
    return 1.0 / (1.0 + np.exp(-v))


def _morph_cat_tension(x, hiddens, Wa1, ba1, Wa2, ba2, Wg1, bg1, Wg2, bg2,
                       W_ih, W_hh, b_ih, b_hh, nat_w, morph_w,
                       morph_src, morph_tgt, step):
    """cat_tension from the 64 morph rows, exact reference math in numpy."""
    needed, inv = np.unique(np.concatenate([morph_src, morph_tgt]),
                            return_inverse=True)
    h_rows = hiddens[needed]  # [R, HID]
    r_n = len(needed)
    comb = np.concatenate(
        [np.broadcast_to(x, (r_n, IN_DIM)), h_rows], axis=1)
    a = np.maximum(comb @ Wa1.T + ba1, 0.0) @ Wa2.T + ba2
    g = np.maximum(comb @ Wg1.T + bg1, 0.0) @ Wg2.T + bg2
    out_r = a - g
    t_r = np.mean(out_r * out_r, axis=-1, keepdims=True)
    mem = np.concatenate([out_r, t_r], axis=1)
    gi = mem @ W_ih.T + b_ih
    gh = h_rows @ W_hh.T + b_hh
    r = _sigmoid(gi[:, :HID] + gh[:, :HID])
    z = _sigmoid(gi[:, HID:2 * HID] + gh[:, HID:2 * HID])
    n_ = np.tanh(gi[:, 2 * HID:] + r * gh[:, 2 * HID:])
    hid_r = (1.0 - z) * n_ + z * h_rows
    if step % 3 == 0:
        hid_r = hid_r @ nat_w.T
    h_src = hid_r[inv[:N_MORPH]]
    h_tgt = hid_r[inv[N_MORPH:]]
    limit = np.einsum("mij,mj->i", morph_w, h_src) / N_MORPH
    colimit = np.einsum("mji,mj->i", morph_w, h_tgt) / N_MORPH
    diff = limit - colimit
    return float(np.mean(diff * diff))


def kernel(**inputs):
    x = np.asarray(inputs["x"], np.float32)
    hiddens = np.asarray(inputs["hiddens"], np.float32)
    Wa1 = np.asarray(inputs["Wa1"], np.float32)
    ba1 = np.asarray(inputs["ba1"], np.float32)
    Wa2 = np.asarray(inputs["Wa2"], np.float32)
    ba2 = np.asarray(inputs["ba2"], np.float32)
    Wg1 = np.asarray(inputs["Wg1"], np.float32)
    bg1 = np.asarray(inputs["bg1"], np.float32)
    Wg2 = np.asarray(inputs["Wg2"], np.float32)
    bg2 = np.asarray(inputs["bg2"], np.float32)
    W_ih = np.asarray(inputs["W_ih"], np.float32)
    W_hh = np.asarray(inputs["W_hh"], np.float32)
    b_ih = np.asarray(inputs["b_ih"], np.float32)
    b_hh = np.asarray(inputs["b_hh"], np.float32)
    nat_w = np.asarray(inputs["nat_w"], np.float32)
    morph_w = np.asarray(inputs["morph_w"], np.float32)
    morph_src = np.asarray(inputs["morph_src"], np.int64)
    morph_tgt = np.asarray(inputs["morph_tgt"], np.int64)
    step = int(np.asarray(inputs["step"]))

    np_dt = _np_data_dt()

    # fold the (cell-independent) x part of layer 1 into the bias
    a1_eff = (ba1.astype(np.float64)
              + x[0].astype(np.float64) @ Wa1[:, :IN_DIM].T.astype(np.float64)
              ).astype(np.float32)
    g1_eff = (bg1.astype(np.float64)
              + x[0].astype(np.float64) @ Wg1[:, :IN_DIM].T.astype(np.float64)
              ).astype(np.float32)
    db = ba2 - bg2

    A1 = Wa1[:, IN_DIM:]  # [MLP_H, HID]
    G1 = Wg1[:, IN_DIM:]
    wA1_h = A1.T.reshape(KC, 128, MLP_H).transpose(1, 0, 2)
    wG1_h = G1.T.reshape(KC, 128, MLP_H).transpose(1, 0, 2)
    wW_h = np.concatenate(
        [wA1_h.reshape(128, KC * MLP_H), wG1_h.reshape(128, KC * MLP_H),
         Wa2.T, -Wg2.T], axis=1).astype(np.float32)          # [128, 768]
    bias_h = np.stack([a1_eff, g1_eff, db, np.zeros(128, np.float32)],
                      axis=1).astype(np.float32)             # [128, 4]

    in_maps = []
    for c in range(N_CORES):
        hs = hiddens[c * SHARD:(c + 1) * SHARD]  # [SHARD, HID]
        hT_h = np.ascontiguousarray(
            hs.T.reshape(KC, 128, SHARD).transpose(1, 0, 2))  # [128,KC,SHARD]
        blob_h = np.concatenate(
            [wW_h, bias_h, hT_h[:, :, :NT].reshape(128, KC * NT)],
            axis=1).astype(np_dt, order="C")                 # [128, 1796]
        hT_rest = np.ascontiguousarray(hT_h[:, :, NT:]).astype(np_dt, order="C")
        in_maps.append({"blob": blob_h, "hT": hT_rest})

    nc = _build_nc()
    profile = bool(int(os.environ.get("CTE_PROFILE", "0")))
    if profile:
        profile = _ensure_ntff_hook()
    bres = run_bass_kernel_spmd(nc, in_maps, list(range(N_CORES)),
                                trace=profile)
    LAST_PROFILE.clear()
    LAST_PROFILE["exec_time_ns"] = bres.exec_time_ns
    LAST_PROFILE["mean_exec_time_ns"] = bres.mean_exec_time_ns
    if bres.instructions_and_trace is not None:
        LAST_PROFILE["trace"] = bres.instructions_and_trace

    V = np.zeros(OUT_DIM, np.float64)
    SQ = 0.0
    E = 0.0
    for r in bres.results:
        out = np.asarray(r["res"], np.float64)
        V += out[:, 0]
        SQ += out[:, 1].sum()
        E += out[0, 2]

    combined_out = (V / E).astype(np.float32)[None, :]
    mean_tension = SQ / (OUT_DIM * N_CELLS)

    cat_tension = _morph_cat_tension(
        x, hiddens, Wa1, ba1, Wa2, ba2, Wg1, bg1, Wg2, bg2,
        W_ih, W_hh, b_ih, b_hh, nat_w, morph_w, morph_src, morph_tgt, step)

    avg_tension = np.float32(mean_tension + 0.1 * cat_tension)
    return combined_out, avg_tension


# revision 37
# speedup vs baseline: 1.0551x; 1.0551x over previous
"""Trainium2 Bass kernel for nn_CategoryTheoryEngine (gnn_message_passing).

reference(...) returns only (combined_out [1,128], avg_tension scalar).

Dead-code analysis of the reference:
  - combined_out = softmax(tension) . out  -> needs per-cell out/tension only
  - avg_tension = mean(tension) + 0.1 * cat_tension
  - cat_tension depends on the GRU'd hidden state at the 64 morph_src/tgt
    rows only (limit/colimit are computed BEFORE the +0.05*diff update).
  - faction sync / debate / hid update only affect the discarded 3rd output.

So the device computes, data-parallel over cells (8 cores x 8192 cells):
  per cell: out = MLP_a(h) - MLP_g(h)  (x-part of layer 1 folded into bias),
  sumsq_c = sum_p out^2, w_c = exp(sumsq_c/128), and reduces
  V = sum_c w_c * out_c  [128],  SQ_p = sum_c out^2[p,c]  [128],  E = sum_c w_c.
Host combines the 8 partial results (combined = V/E) and computes the 64
morph rows' GRU -> limit/colimit -> cat_tension in numpy (0.1% of cells).
"""

import os
import sys
import types
from contextlib import ExitStack

import numpy as np

import concourse.bass as bass
import concourse.tile as tile
from concourse import mybir
from concourse.bass_utils import run_bass_kernel_spmd


def _ensure_ntff_hook():
    """The agent image's antenv lacks axon_hooks; recreate it so
    run_bass_kernel_spmd(trace=True) can NTFF-profile through axon."""
    try:
        from antenv.axon_hooks import get_axon_ntff_profile_hook  # noqa: F401

        return True
    except ImportError:
        pass
    try:
        import antenv

        if "/root/.axon_site" not in sys.path:
            sys.path.insert(0, "/root/.axon_site")
        from trn_agent_boot.trn_boot import _ntff_profile_via_ctypes

        mod = types.ModuleType("antenv.axon_hooks")
        state = {"hook": None}
        mod.set_axon_ntff_profile_hook = lambda h: state.__setitem__("hook", h)
        mod.get_axon_ntff_profile_hook = lambda: state["hook"]
        sys.modules["antenv.axon_hooks"] = mod
        antenv.axon_hooks = mod
        mod.set_axon_ntff_profile_hook(
            _ntff_profile_via_ctypes("/opt/axon/libaxon_pjrt.so"))
        return True
    except Exception as e:  # profiling is best-effort only
        print(f"[kernel] ntff hook install failed: {e}")
        return False

def _patch_tail_drain():
    """The stock kernel-tail emits ONE SP Drain waiting every proc's final
    semaphore tick; with 3 engines + several DMA queues that exceeds the
    Drain instruction's sync-wait slots and walrus refuses to codegen.
    Split the waits across several Drain instructions (<=4 waits each)."""
    if getattr(tile.TileContext, "_cte_split_drain", False):
        return
    from concourse.vector_clock import ScopedClock, VectorClock

    def _drain_and_barrier(self, tick_clock, wait_clock):
        gc = tick_clock.global_clock
        n = len(gc)
        procs = [i for i in range(n) if gc[i] > 0]
        groups = [[p] for p in procs] or [[]]
        for grp in groups:
            vec = [gc[i] if i in grp else 0 for i in range(n)]
            drain_inst = self.nc.sync.drain()
            wait_clock.add_sem_waits(
                drain_inst.ins, ScopedClock({None: VectorClock(vec)}))
        self.nc.all_engine_barrier()
        assert self.sems is not None
        popped = self.nc._tile_sem_poison_stack.pop()
        assert popped is self._sem_poison
        self.nc.clear_and_free_semaphores(
            list(self.sems.allocated().values()))
        self.nc.all_engine_barrier()

    tile.TileContext._drain_and_barrier = _drain_and_barrier
    tile.TileContext._cte_split_drain = True


_patch_tail_drain()

N_CELLS, IN_DIM, HID, OUT_DIM, MLP_H, N_MORPH = 65536, 128, 256, 128, 128, 32
N_CORES = 8
SHARD = N_CELLS // N_CORES  # 8192
NT = 512                    # cells per on-chip tile
NTILES = SHARD // NT        # 16
KC = HID // 128             # 2 contraction chunks for layer 1

F32 = mybir.dt.float32
F32R = mybir.dt.float32r
BF16 = mybir.dt.bfloat16

# 'f32r': fp32 storage, full-rate fp32r matmuls.  'bf16': bf16 storage+matmuls
# (halves DMA).  'f32': plain fp32 matmuls (4x slower, debug only).
MM_MODE = os.environ.get("CTE_MM_MODE", "f32r")

LAST_PROFILE = {}

_nc_cache = {}


def _data_dt():
    if MM_MODE == "bf16":
        return BF16
    if MM_MODE == "f32r":
        return F32R
    return F32


def _np_data_dt():
    if MM_MODE == "bf16":
        import ml_dtypes

        return ml_dtypes.bfloat16
    return np.float32


def _mm(ap):
    return ap


def _build_nc():
    if MM_MODE in _nc_cache:
        return _nc_cache[MM_MODE]

    dt = _data_dt()
    AF = mybir.ActivationFunctionType
    ALU = mybir.AluOpType

    nc = bass.Bass()
    # tiles 1..NTILES-1 of the hiddens shard (tile 0 rides in the const blob)
    hT = nc.declare_dram_parameter("hT", [128, KC, SHARD - NT], dt, False)
    # one const blob -> ONE DMA -> one semaphore for everything the first
    # matmul needs (PE matmul tolerates only a single sync wait):
    # free-dim layout: [0:768] weights (A1T k0,k1 | G1T k0,k1 | Wa2T | -Wg2T),
    # [768:771] bias cols (f32 bits: ba1_eff, bg1_eff, db), [771:772] pad,
    # [772:1796] tile-0 h_t (2 chunks x NT)
    CB_W, CB_B, CB_H0, CB_TOT = 0, 768, 772, 772 + KC * NT
    blob = nc.declare_dram_parameter("blob", [128, CB_TOT], dt, False)
    res = nc.declare_dram_parameter("res", [128, 4], F32, True)

    with tile.TileContext(nc) as tc, ExitStack() as ctx:
        consts = ctx.enter_context(tc.tile_pool(name="consts", bufs=1))
        loads = ctx.enter_context(tc.tile_pool(name="loads", bufs=1))
        work = ctx.enter_context(tc.tile_pool(name="work", bufs=6))
        accs = ctx.enter_context(tc.tile_pool(name="accs", bufs=1))
        wsb = ctx.enter_context(tc.tile_pool(name="wsb", bufs=NTILES))
        outs = ctx.enter_context(tc.tile_pool(name="outs", bufs=NTILES))
        # PSUM bank budget (8 banks): p_l1 x1, tp x1, p2 x2, wb x4.
        # wb needs depth 4 so its slot is provably free at schedule time --
        # its reader is the DVE product whose completion PE never observes,
        # and a PE matmul can carry only one sync wait.
        ps1 = ctx.enter_context(tc.tile_pool(name="ps1", bufs=1, space="PSUM"))
        ps2 = ctx.enter_context(tc.tile_pool(name="ps2", bufs=2, space="PSUM"))
        ps4 = ctx.enter_context(tc.tile_pool(name="ps4", bufs=4, space="PSUM"))

        blob_sb = consts.tile([128, CB_TOT], dt)
        nc.gpsimd.dma_start(out=blob_sb[:], in_=blob[:])
        wA1_sb = [blob_sb[:, k * MLP_H:(k + 1) * MLP_H] for k in range(KC)]
        wG1_sb = [blob_sb[:, (KC + k) * MLP_H:(KC + k + 1) * MLP_H]
                  for k in range(KC)]
        wA2_sb = blob_sb[:, 4 * MLP_H:5 * MLP_H]
        wG2n_sb = blob_sb[:, 5 * MLP_H:6 * MLP_H]
        bias_sb = blob_sb[:, CB_B:CB_B + 3].bitcast(F32)
        h0_sb = [blob_sb[:, CB_H0 + k * NT:CB_H0 + (k + 1) * NT]
                 for k in range(KC)]

        # ones produced on ACT (DVE memset cannot write float32r, and ACT
        # keeps the consumers' deps ACT-local): copy(x*0 + 1) = 1
        ones_col = consts.tile([128, 1], dt)
        nc.scalar.activation(out=ones_col[:], in_=bias_sb[:, 0:1],
                             func=AF.Copy, scale=0.0, bias=1.0)
        ones_row = consts.tile([1, 128], dt)
        nc.scalar.activation(out=ones_row[:], in_=blob_sb[0:1, 0:MLP_H],
                             func=AF.Copy, scale=0.0, bias=1.0)
        # zero produced on ACT (not DVE) so the exp's bias dep stays ACT-local;
        # doubles as the ACT pre-touch of the const-blob DMA
        zero1 = consts.tile([1, 1], F32)
        nc.scalar.mul(zero1[:], bias_sb[0:1, 0:1], 0.0)

        # scratch target for DVE guard copies (absorbs PE waits so the
        # product TensorTensor carries only its ACT wait)
        gwp = ctx.enter_context(tc.tile_pool(name="gwp", bufs=8))
        vacc = accs.tile([128, NTILES], F32)
        sqacc = accs.tile([128, NTILES], F32)
        eacc = accs.tile([1, NTILES], F32)

        # the 15 remaining tiles arrive in a few chunked DMAs (pipelining
        # vs. startup bubble; consumers slice the big resident tiles)
        NCHUNK = 5
        per = 3 * NT  # 3 tiles per chunk
        h_chunks = []
        for h in range(NCHUNK):
            lo = h * per
            hi = min(SHARD - NT, lo + per)
            ht_big = loads.tile([128, KC, per], dt, tag=f"ht_big{h}")
            nc.gpsimd.dma_start(out=ht_big[:, :, :hi - lo], in_=hT[:, :, lo:hi])
            h_chunks.append(ht_big)

        prev_prod = None
        for i in range(NTILES):
            if i == 0:
                h_k = h0_sb
            else:
                h, off = (i - 1) // 3, ((i - 1) % 3) * NT
                h_k = [h_chunks[h][:, k, off:off + NT] for k in range(KC)]

            # layer 1: a and g sequentially through one single-bank tag
            p1a = ps2.tile([128, NT], F32, tag="p_l1")
            if i > 0 and (i - 1) % 3 == 0:
                # new input chunk: absorb its DMA wait on a standalone
                # ldweights so the first matmul keeps a single sync wait
                nc.tensor.ldweights(h_k[0][:, 0:32].bitcast(BF16))
            for k in range(KC):
                nc.tensor.matmul(
                    p1a[:], _mm(wA1_sb[k]), _mm(h_k[k]),
                    start=(k == 0), stop=(k == KC - 1),
                )
            acts_a = work.tile([128, NT], dt)
            nc.scalar.activation(
                out=acts_a[:], in_=p1a[:], func=AF.Relu,
                bias=bias_sb[:, 0:1], scale=1.0,
            )
            p1g = ps2.tile([128, NT], F32, tag="p_l1")
            for k in range(KC):
                nc.tensor.matmul(
                    p1g[:], _mm(wG1_sb[k]), _mm(h_k[k]),
                    start=(k == 0), stop=(k == KC - 1),
                )
            acts_g = work.tile([128, NT], dt)
            nc.scalar.activation(
                out=acts_g[:], in_=p1g[:], func=AF.Relu,
                bias=bias_sb[:, 1:2], scale=1.0,
            )

            # layer 2: p2 = a' @ Wa2.T - g' @ Wg2.T  (= out - db)
            p2 = ps1.tile([128, NT], F32, tag="p2")
            nc.tensor.matmul(p2[:], _mm(wA2_sb), _mm(acts_a[:]),
                             start=True, stop=False)
            nc.tensor.matmul(p2[:], _mm(wG2n_sb), _mm(acts_g[:]),
                             start=False, stop=True)

            # out = p2 + db, materialized in SBUF (engines may read at most
            # one PSUM operand per instruction)
            out_sb = outs.tile([128, NT], F32)
            nc.scalar.activation(
                out=out_sb[:], in_=p2[:], func=AF.Identity,
                bias=bias_sb[:, 2:3], scale=1.0,
            )

            # sq = out^2 ; ACT accumulator gives sum over cells per partition
            sq = work.tile([128, NT], dt)
            nc.scalar.activation(
                out=sq[:], in_=out_sb[:], func=AF.Square,
                accum_out=sqacc[:, i : i + 1],
            )

            # tension*128 per cell: sum over partitions via ones-matmul
            tp = ps1.tile([1, NT], F32)
            nc.tensor.matmul(tp[:], _mm(ones_col[:]), _mm(sq[:]),
                             start=True, stop=True)

            # w = exp(t) ; accumulate sum of w
            w_sb = wsb.tile([1, NT], dt)
            nc.scalar.activation(
                out=w_sb[:], in_=tp[:], func=AF.Exp,
                bias=zero1[0:1, 0:1], scale=1.0 / OUT_DIM,
                accum_out=eacc[:, i : i + 1],
            )

            # broadcast w across partitions (rank-1 matmul). The ldweights
            # guard reads the previous tile's DVE product so PE observes the
            # DVE tick that releases this wb slot -- the real matmul then
            # carries only its ACT wait (PE matmuls support a single sync
            # wait). The stray weight load is overwritten by the matmul's
            # own self-loading LDWEIGHTS.
            wb = ps4.tile([128, NT], F32)
            if prev_prod is not None:
                nc.tensor.ldweights(prev_prod[:, 0:32].bitcast(BF16))
            nc.tensor.matmul(wb[:], _mm(ones_row[:]), _mm(w_sb[:]),
                             start=True, stop=True)

            # V partial: sum_c w_c * out[:, c]
            prod = work.tile([128, NT], F32)
            dve_gw = gwp.tile([1, 1], F32, tag="gw1")
            nc.vector.tensor_copy(out=dve_gw[0:1, 0:1], in_=wb[0:1, 0:1])
            dve_gw2 = gwp.tile([1, 1], F32, tag="gw2")
            nc.vector.tensor_copy(out=dve_gw2[0:1, 0:1], in_=out_sb[0:1, 0:1])
            nc.vector.tensor_mul(out=prod[:], in0=out_sb[:], in1=wb[:])
            nc.vector.tensor_reduce(
                out=vacc[:, i : i + 1], in_=prod[:],
                axis=mybir.AxisListType.X, op=ALU.add,
            )
            prev_prod = prod

        outsb = consts.tile([128, 4], F32)
        nc.vector.memset(outsb[:], 0.0)
        nc.vector.tensor_reduce(out=outsb[:, 0:1], in_=vacc[:],
                                axis=mybir.AxisListType.X, op=mybir.AluOpType.add)
        nc.vector.tensor_reduce(out=outsb[:, 1:2], in_=sqacc[:],
                                axis=mybir.AxisListType.X, op=mybir.AluOpType.add)
        nc.vector.tensor_reduce(out=outsb[0:1, 2:3], in_=eacc[:],
                                axis=mybir.AxisListType.X, op=mybir.AluOpType.add)
        nc.sync.dma_start(out=res[:], in_=outsb[:])

    nc.finalize()
    _nc_cache[MM_MODE] = nc
    return nc


def _sigmoid(v):
    return 1.0 / (1.0 + np.exp(-v))


def _morph_cat_tension(x, hiddens, Wa1, ba1, Wa2, ba2, Wg1, bg1, Wg2, bg2,
                       W_ih, W_hh, b_ih, b_hh, nat_w, morph_w,
                       morph_src, morph_tgt, step):
    """cat_tension from the 64 morph rows, exact reference math in numpy."""
    needed, inv = np.unique(np.concatenate([morph_src, morph_tgt]),
                            return_inverse=True)
    h_rows = hiddens[needed]  # [R, HID]
    r_n = len(needed)
    comb = np.concatenate(
        [np.broadcast_to(x, (r_n, IN_DIM)), h_rows], axis=1)
    a = np.maximum(comb @ Wa1.T + ba1, 0.0) @ Wa2.T + ba2
    g = np.maximum(comb @ Wg1.T + bg1, 0.0) @ Wg2.T + bg2
    out_r = a - g
    t_r = np.mean(out_r * out_r, axis=-1, keepdims=True)
    mem = np.concatenate([out_r, t_r], axis=1)
    gi = mem @ W_ih.T + b_ih
    gh = h_rows @ W_hh.T + b_hh
    r = _sigmoid(gi[:, :HID] + gh[:, :HID])
    z = _sigmoid(gi[:, HID:2 * HID] + gh[:, HID:2 * HID])
    n_ = np.tanh(gi[:, 2 * HID:] + r * gh[:, 2 * HID:])
    hid_r = (1.0 - z) * n_ + z * h_rows
    if step % 3 == 0:
        hid_r = hid_r @ nat_w.T
    h_src = hid_r[inv[:N_MORPH]]
    h_tgt = hid_r[inv[N_MORPH:]]
    limit = np.einsum("mij,mj->i", morph_w, h_src) / N_MORPH
    colimit = np.einsum("mji,mj->i", morph_w, h_tgt) / N_MORPH
    diff = limit - colimit
    return float(np.mean(diff * diff))


def kernel(**inputs):
    x = np.asarray(inputs["x"], np.float32)
    hiddens = np.asarray(inputs["hiddens"], np.float32)
    Wa1 = np.asarray(inputs["Wa1"], np.float32)
    ba1 = np.asarray(inputs["ba1"], np.float32)
    Wa2 = np.asarray(inputs["Wa2"], np.float32)
    ba2 = np.asarray(inputs["ba2"], np.float32)
    Wg1 = np.asarray(inputs["Wg1"], np.float32)
    bg1 = np.asarray(inputs["bg1"], np.float32)
    Wg2 = np.asarray(inputs["Wg2"], np.float32)
    bg2 = np.asarray(inputs["bg2"], np.float32)
    W_ih = np.asarray(inputs["W_ih"], np.float32)
    W_hh = np.asarray(inputs["W_hh"], np.float32)
    b_ih = np.asarray(inputs["b_ih"], np.float32)
    b_hh = np.asarray(inputs["b_hh"], np.float32)
    nat_w = np.asarray(inputs["nat_w"], np.float32)
    morph_w = np.asarray(inputs["morph_w"], np.float32)
    morph_src = np.asarray(inputs["morph_src"], np.int64)
    morph_tgt = np.asarray(inputs["morph_tgt"], np.int64)
    step = int(np.asarray(inputs["step"]))

    np_dt = _np_data_dt()

    # fold the (cell-independent) x part of layer 1 into the bias
    a1_eff = (ba1.astype(np.float64)
              + x[0].astype(np.float64) @ Wa1[:, :IN_DIM].T.astype(np.float64)
              ).astype(np.float32)
    g1_eff = (bg1.astype(np.float64)
              + x[0].astype(np.float64) @ Wg1[:, :IN_DIM].T.astype(np.float64)
              ).astype(np.float32)
    db = ba2 - bg2

    A1 = Wa1[:, IN_DIM:]  # [MLP_H, HID]
    G1 = Wg1[:, IN_DIM:]
    wA1_h = A1.T.reshape(KC, 128, MLP_H).transpose(1, 0, 2)
    wG1_h = G1.T.reshape(KC, 128, MLP_H).transpose(1, 0, 2)
    wW_h = np.concatenate(
        [wA1_h.reshape(128, KC * MLP_H), wG1_h.reshape(128, KC * MLP_H),
         Wa2.T, -Wg2.T], axis=1).astype(np.float32)          # [128, 768]
    bias_h = np.stack([a1_eff, g1_eff, db, np.zeros(128, np.float32)],
                      axis=1).astype(np.float32)             # [128, 4]

    in_maps = []
    for c in range(N_CORES):
        hs = hiddens[c * SHARD:(c + 1) * SHARD]  # [SHARD, HID]
        hT_h = np.ascontiguousarray(
            hs.T.reshape(KC, 128, SHARD).transpose(1, 0, 2))  # [128,KC,SHARD]
        blob_h = np.concatenate(
            [wW_h, bias_h, hT_h[:, :, :NT].reshape(128, KC * NT)],
            axis=1).astype(np_dt, order="C")                 # [128, 1796]
        hT_rest = np.ascontiguousarray(hT_h[:, :, NT:]).astype(np_dt, order="C")
        in_maps.append({"blob": blob_h, "hT": hT_rest})

    nc = _build_nc()
    profile = bool(int(os.environ.get("CTE_PROFILE", "0")))
    if profile:
        profile = _ensure_ntff_hook()
    bres = run_bass_kernel_spmd(nc, in_maps, list(range(N_CORES)),
                                trace=profile)
    LAST_PROFILE.clear()
    LAST_PROFILE["exec_time_ns"] = bres.exec_time_ns
    LAST_PROFILE["mean_exec_time_ns"] = bres.mean_exec_time_ns
    if bres.instructions_and_trace is not None:
        LAST_PROFILE["trace"] = bres.instructions_and_trace

    V = np.zeros(OUT_DIM, np.float64)
    SQ = 0.0
    E = 0.0
    for r in bres.results:
        out = np.asarray(r["res"], np.float64)
        V += out[:, 0]
        SQ += out[:, 1].sum()
        E += out[0, 2]

    combined_out = (V / E).astype(np.float32)[None, :]
    mean_tension = SQ / (OUT_DIM * N_CELLS)

    cat_tension = _morph_cat_tension(
        x, hiddens, Wa1, ba1, Wa2, ba2, Wg1, bg1, Wg2, bg2,
        W_ih, W_hh, b_ih, b_hh, nat_w, morph_w, morph_src, morph_tgt, step)

    avg_tension = np.float32(mean_tension + 0.1 * cat_tension)
    return combined_out, avg_tension


# revision 38
# speedup vs baseline: 1.0924x; 1.0354x over previous
"""Trainium2 Bass kernel for nn_CategoryTheoryEngine (gnn_message_passing).

reference(...) returns only (combined_out [1,128], avg_tension scalar).

Dead-code analysis of the reference:
  - combined_out = softmax(tension) . out  -> needs per-cell out/tension only
  - avg_tension = mean(tension) + 0.1 * cat_tension
  - cat_tension depends on the GRU'd hidden state at the 64 morph_src/tgt
    rows only (limit/colimit are computed BEFORE the +0.05*diff update).
  - faction sync / debate / hid update only affect the discarded 3rd output.

So the device computes, data-parallel over cells (8 cores x 8192 cells):
  per cell: out = MLP_a(h) - MLP_g(h)  (x-part of layer 1 folded into bias),
  sumsq_c = sum_p out^2, w_c = exp(sumsq_c/128), and reduces
  V = sum_c w_c * out_c  [128],  SQ_p = sum_c out^2[p,c]  [128],  E = sum_c w_c.
Host combines the 8 partial results (combined = V/E) and computes the 64
morph rows' GRU -> limit/colimit -> cat_tension in numpy (0.1% of cells).
"""

import os
import sys
import types
from contextlib import ExitStack

import numpy as np

import concourse.bass as bass
import concourse.tile as tile
from concourse import mybir
from concourse.bass_utils import run_bass_kernel_spmd


def _ensure_ntff_hook():
    """The agent image's antenv lacks axon_hooks; recreate it so
    run_bass_kernel_spmd(trace=True) can NTFF-profile through axon."""
    try:
        from antenv.axon_hooks import get_axon_ntff_profile_hook  # noqa: F401

        return True
    except ImportError:
        pass
    try:
        import antenv

        if "/root/.axon_site" not in sys.path:
            sys.path.insert(0, "/root/.axon_site")
        from trn_agent_boot.trn_boot import _ntff_profile_via_ctypes

        mod = types.ModuleType("antenv.axon_hooks")
        state = {"hook": None}
        mod.set_axon_ntff_profile_hook = lambda h: state.__setitem__("hook", h)
        mod.get_axon_ntff_profile_hook = lambda: state["hook"]
        sys.modules["antenv.axon_hooks"] = mod
        antenv.axon_hooks = mod
        mod.set_axon_ntff_profile_hook(
            _ntff_profile_via_ctypes("/opt/axon/libaxon_pjrt.so"))
        return True
    except Exception as e:  # profiling is best-effort only
        print(f"[kernel] ntff hook install failed: {e}")
        return False

def _patch_tail_drain():
    """The stock kernel-tail emits ONE SP Drain waiting every proc's final
    semaphore tick; with 3 engines + several DMA queues that exceeds the
    Drain instruction's sync-wait slots and walrus refuses to codegen.
    Split the waits across several Drain instructions (<=4 waits each)."""
    if getattr(tile.TileContext, "_cte_split_drain", False):
        return
    from concourse.vector_clock import ScopedClock, VectorClock

    def _drain_and_barrier(self, tick_clock, wait_clock):
        gc = tick_clock.global_clock
        n = len(gc)
        procs = [i for i in range(n) if gc[i] > 0]
        groups = [[p] for p in procs] or [[]]
        for grp in groups:
            vec = [gc[i] if i in grp else 0 for i in range(n)]
            drain_inst = self.nc.sync.drain()
            wait_clock.add_sem_waits(
                drain_inst.ins, ScopedClock({None: VectorClock(vec)}))
        self.nc.all_engine_barrier()
        assert self.sems is not None
        popped = self.nc._tile_sem_poison_stack.pop()
        assert popped is self._sem_poison
        self.nc.clear_and_free_semaphores(
            list(self.sems.allocated().values()))
        self.nc.all_engine_barrier()

    tile.TileContext._drain_and_barrier = _drain_and_barrier
    tile.TileContext._cte_split_drain = True


_patch_tail_drain()

N_CELLS, IN_DIM, HID, OUT_DIM, MLP_H, N_MORPH = 65536, 128, 256, 128, 128, 32
N_CORES = 8
SHARD = N_CELLS // N_CORES  # 8192
NT = 512                    # cells per on-chip tile
NTILES = SHARD // NT        # 16
KC = HID // 128             # 2 contraction chunks for layer 1

F32 = mybir.dt.float32
F32R = mybir.dt.float32r
BF16 = mybir.dt.bfloat16

# 'f32r': fp32 storage, full-rate fp32r matmuls.  'bf16': bf16 storage+matmuls
# (halves DMA).  'f32': plain fp32 matmuls (4x slower, debug only).
MM_MODE = os.environ.get("CTE_MM_MODE", "f32r")

LAST_PROFILE = {}

_nc_cache = {}


def _data_dt():
    if MM_MODE == "bf16":
        return BF16
    if MM_MODE == "f32r":
        return F32R
    return F32


def _np_data_dt():
    if MM_MODE == "bf16":
        import ml_dtypes

        return ml_dtypes.bfloat16
    return np.float32


def _mm(ap):
    return ap


def _build_nc():
    if MM_MODE in _nc_cache:
        return _nc_cache[MM_MODE]

    dt = _data_dt()
    AF = mybir.ActivationFunctionType
    ALU = mybir.AluOpType

    nc = bass.Bass()
    # tiles 1..NTILES-1 of the hiddens shard (tile 0 rides in the const blob)
    hT = nc.declare_dram_parameter("hT", [128, KC, SHARD - NT], dt, False)
    # one const blob -> ONE DMA -> one semaphore for everything the first
    # matmul needs (PE matmul tolerates only a single sync wait):
    # free-dim layout: [0:768] weights (A1T k0,k1 | G1T k0,k1 | Wa2T | -Wg2T),
    # [768:771] bias cols (f32 bits: ba1_eff, bg1_eff, db), [771:772] pad,
    # [772:1796] tile-0 h_t (2 chunks x NT)
    CB_W, CB_B, CB_H0, CB_TOT = 0, 768, 772, 772 + KC * NT
    blob = nc.declare_dram_parameter("blob", [128, CB_TOT], dt, False)
    res = nc.declare_dram_parameter("res", [128, 4], F32, True)

    with tile.TileContext(nc) as tc, ExitStack() as ctx:
        consts = ctx.enter_context(tc.tile_pool(name="consts", bufs=1))
        loads = ctx.enter_context(tc.tile_pool(name="loads", bufs=1))
        work = ctx.enter_context(tc.tile_pool(name="work", bufs=6))
        accs = ctx.enter_context(tc.tile_pool(name="accs", bufs=1))
        wsb = ctx.enter_context(tc.tile_pool(name="wsb", bufs=NTILES))
        outs = ctx.enter_context(tc.tile_pool(name="outs", bufs=NTILES))
        # PSUM bank budget (8 banks): p_l1 x1, tp x1, p2 x2, wb x4.
        # wb needs depth 4 so its slot is provably free at schedule time --
        # its reader is the DVE product whose completion PE never observes,
        # and a PE matmul can carry only one sync wait.
        ps1 = ctx.enter_context(tc.tile_pool(name="ps1", bufs=1, space="PSUM"))
        ps2 = ctx.enter_context(tc.tile_pool(name="ps2", bufs=4, space="PSUM"))
        ps4 = ctx.enter_context(tc.tile_pool(name="ps4", bufs=2, space="PSUM"))

        blob_sb = consts.tile([128, CB_TOT], dt)
        nc.gpsimd.dma_start(out=blob_sb[:], in_=blob[:])
        wA1_sb = [blob_sb[:, k * MLP_H:(k + 1) * MLP_H] for k in range(KC)]
        wG1_sb = [blob_sb[:, (KC + k) * MLP_H:(KC + k + 1) * MLP_H]
                  for k in range(KC)]
        wA2_sb = blob_sb[:, 4 * MLP_H:5 * MLP_H]
        wG2n_sb = blob_sb[:, 5 * MLP_H:6 * MLP_H]
        bias_sb = blob_sb[:, CB_B:CB_B + 3].bitcast(F32)
        h0_sb = [blob_sb[:, CB_H0 + k * NT:CB_H0 + (k + 1) * NT]
                 for k in range(KC)]

        # ones produced on ACT (DVE memset cannot write float32r, and ACT
        # keeps the consumers' deps ACT-local): copy(x*0 + 1) = 1
        ones_col = consts.tile([128, 1], dt)
        nc.scalar.activation(out=ones_col[:], in_=bias_sb[:, 0:1],
                             func=AF.Copy, scale=0.0, bias=1.0)
        ones_row = consts.tile([1, 128], dt)
        nc.scalar.activation(out=ones_row[:], in_=blob_sb[0:1, 0:MLP_H],
                             func=AF.Copy, scale=0.0, bias=1.0)
        # zero produced on ACT (not DVE) so the exp's bias dep stays ACT-local;
        # doubles as the ACT pre-touch of the const-blob DMA
        zero1 = consts.tile([1, 1], F32)
        nc.scalar.mul(zero1[:], bias_sb[0:1, 0:1], 0.0)

        # scratch target for DVE guard copies (absorbs PE waits so the
        # product TensorTensor carries only its ACT wait)
        gwp = ctx.enter_context(tc.tile_pool(name="gwp", bufs=8))
        vacc = accs.tile([128, NTILES], F32)
        sqacc = accs.tile([128, NTILES], F32)
        eacc = accs.tile([1, NTILES], F32)

        # the 15 remaining tiles arrive in a few chunked DMAs (pipelining
        # vs. startup bubble; consumers slice the big resident tiles)
        NCHUNK = 5
        per = 3 * NT  # 3 tiles per chunk
        h_chunks = []
        for h in range(NCHUNK):
            lo = h * per
            hi = min(SHARD - NT, lo + per)
            ht_big = loads.tile([128, KC, per], dt, tag=f"ht_big{h}")
            nc.gpsimd.dma_start(out=ht_big[:, :, :hi - lo], in_=hT[:, :, lo:hi])
            h_chunks.append(ht_big)

        prev_prod = None
        for i in range(NTILES):
            if i == 0:
                h_k = h0_sb
            else:
                h, off = (i - 1) // 3, ((i - 1) % 3) * NT
                h_k = [h_chunks[h][:, k, off:off + NT] for k in range(KC)]

            # layer 1: a and g sequentially through one single-bank tag
            p1a = ps2.tile([128, NT], F32, tag="p_l1")
            if i > 0 and (i - 1) % 3 == 0:
                # new input chunk: absorb its DMA wait on a standalone
                # ldweights so the first matmul keeps a single sync wait
                nc.tensor.ldweights(h_k[0][:, 0:32].bitcast(BF16))
            for k in range(KC):
                nc.tensor.matmul(
                    p1a[:], _mm(wA1_sb[k]), _mm(h_k[k]),
                    start=(k == 0), stop=(k == KC - 1),
                )
            acts_a = work.tile([128, NT], dt)
            nc.scalar.activation(
                out=acts_a[:], in_=p1a[:], func=AF.Relu,
                bias=bias_sb[:, 0:1], scale=1.0,
            )
            p1g = ps2.tile([128, NT], F32, tag="p_l1")
            for k in range(KC):
                nc.tensor.matmul(
                    p1g[:], _mm(wG1_sb[k]), _mm(h_k[k]),
                    start=(k == 0), stop=(k == KC - 1),
                )
            acts_g = work.tile([128, NT], dt)
            nc.scalar.activation(
                out=acts_g[:], in_=p1g[:], func=AF.Relu,
                bias=bias_sb[:, 1:2], scale=1.0,
            )

            # layer 2: p2 = a' @ Wa2.T - g' @ Wg2.T  (= out - db)
            p2 = ps1.tile([128, NT], F32, tag="p2")
            nc.tensor.matmul(p2[:], _mm(wA2_sb), _mm(acts_a[:]),
                             start=True, stop=False)
            nc.tensor.matmul(p2[:], _mm(wG2n_sb), _mm(acts_g[:]),
                             start=False, stop=True)

            # out = p2 + db, materialized in SBUF (engines may read at most
            # one PSUM operand per instruction)
            out_sb = outs.tile([128, NT], F32)
            nc.scalar.activation(
                out=out_sb[:], in_=p2[:], func=AF.Identity,
                bias=bias_sb[:, 2:3], scale=1.0,
            )

            # sq = out^2 ; ACT accumulator gives sum over cells per partition
            sq = work.tile([128, NT], dt)
            nc.scalar.activation(
                out=sq[:], in_=out_sb[:], func=AF.Square,
                accum_out=sqacc[:, i : i + 1],
            )

            # tension*128 per cell: sum over partitions via ones-matmul
            tp = ps1.tile([1, NT], F32)
            nc.tensor.matmul(tp[:], _mm(ones_col[:]), _mm(sq[:]),
                             start=True, stop=True)

            # w = exp(t) ; accumulate sum of w
            w_sb = wsb.tile([1, NT], dt)
            nc.scalar.activation(
                out=w_sb[:], in_=tp[:], func=AF.Exp,
                bias=zero1[0:1, 0:1], scale=1.0 / OUT_DIM,
                accum_out=eacc[:, i : i + 1],
            )

            # broadcast w across partitions (rank-1 matmul). The ldweights
            # guard reads the previous tile's DVE product so PE observes the
            # DVE tick that releases this wb slot -- the real matmul then
            # carries only its ACT wait (PE matmuls support a single sync
            # wait). The stray weight load is overwritten by the matmul's
            # own self-loading LDWEIGHTS.
            wb = ps4.tile([128, NT], F32)
            if prev_prod is not None:
                nc.tensor.ldweights(prev_prod[:, 0:32].bitcast(BF16))
            nc.tensor.matmul(wb[:], _mm(ones_row[:]), _mm(w_sb[:]),
                             start=True, stop=True)

            # V partial: sum_c w_c * out[:, c]
            prod = work.tile([128, NT], F32)
            dve_gw = gwp.tile([1, 1], F32, tag="gw1")
            nc.vector.tensor_copy(out=dve_gw[0:1, 0:1], in_=wb[0:1, 0:1])
            dve_gw2 = gwp.tile([1, 1], F32, tag="gw2")
            nc.vector.tensor_copy(out=dve_gw2[0:1, 0:1], in_=out_sb[0:1, 0:1])
            nc.vector.tensor_mul(out=prod[:], in0=out_sb[:], in1=wb[:])
            nc.vector.tensor_reduce(
                out=vacc[:, i : i + 1], in_=prod[:],
                axis=mybir.AxisListType.X, op=ALU.add,
            )
            prev_prod = prod

        outsb = consts.tile([128, 4], F32)
        nc.vector.memset(outsb[:], 0.0)
        nc.vector.tensor_reduce(out=outsb[:, 0:1], in_=vacc[:],
                                axis=mybir.AxisListType.X, op=mybir.AluOpType.add)
        nc.vector.tensor_reduce(out=outsb[:, 1:2], in_=sqacc[:],
                                axis=mybir.AxisListType.X, op=mybir.AluOpType.add)
        nc.vector.tensor_reduce(out=outsb[0:1, 2:3], in_=eacc[:],
                                axis=mybir.AxisListType.X, op=mybir.AluOpType.add)
        nc.sync.dma_start(out=res[:], in_=outsb[:])

    nc.finalize()
    _nc_cache[MM_MODE] = nc
    return nc


def _sigmoid(v):
    return 1.0 / (1.0 + np.exp(-v))


def _morph_cat_tension(x, hiddens, Wa1, ba1, Wa2, ba2, Wg1, bg1, Wg2, bg2,
                       W_ih, W_hh, b_ih, b_hh, nat_w, morph_w,
                       morph_src, morph_tgt, step):
    """cat_tension from the 64 morph rows, exact reference math in numpy."""
    needed, inv = np.unique(np.concatenate([morph_src, morph_tgt]),
                            return_inverse=True)
    h_rows = hiddens[needed]  # [R, HID]
    r_n = len(needed)
    comb = np.concatenate(
        [np.broadcast_to(x, (r_n, IN_DIM)), h_rows], axis=1)
    a = np.maximum(comb @ Wa1.T + ba1, 0.0) @ Wa2.T + ba2
    g = np.maximum(comb @ Wg1.T + bg1, 0.0) @ Wg2.T + bg2
    out_r = a - g
    t_r = np.mean(out_r * out_r, axis=-1, keepdims=True)
    mem = np.concatenate([out_r, t_r], axis=1)
    gi = mem @ W_ih.T + b_ih
    gh = h_rows @ W_hh.T + b_hh
    r = _sigmoid(gi[:, :HID] + gh[:, :HID])
    z = _sigmoid(gi[:, HID:2 * HID] + gh[:, HID:2 * HID])
    n_ = np.tanh(gi[:, 2 * HID:] + r * gh[:, 2 * HID:])
    hid_r = (1.0 - z) * n_ + z * h_rows
    if step % 3 == 0:
        hid_r = hid_r @ nat_w.T
    h_src = hid_r[inv[:N_MORPH]]
    h_tgt = hid_r[inv[N_MORPH:]]
    limit = np.einsum("mij,mj->i", morph_w, h_src) / N_MORPH
    colimit = np.einsum("mji,mj->i", morph_w, h_tgt) / N_MORPH
    diff = limit - colimit
    return float(np.mean(diff * diff))


def kernel(**inputs):
    x = np.asarray(inputs["x"], np.float32)
    hiddens = np.asarray(inputs["hiddens"], np.float32)
    Wa1 = np.asarray(inputs["Wa1"], np.float32)
    ba1 = np.asarray(inputs["ba1"], np.float32)
    Wa2 = np.asarray(inputs["Wa2"], np.float32)
    ba2 = np.asarray(inputs["ba2"], np.float32)
    Wg1 = np.asarray(inputs["Wg1"], np.float32)
    bg1 = np.asarray(inputs["bg1"], np.float32)
    Wg2 = np.asarray(inputs["Wg2"], np.float32)
    bg2 = np.asarray(inputs["bg2"], np.float32)
    W_ih = np.asarray(inputs["W_ih"], np.float32)
    W_hh = np.asarray(inputs["W_hh"], np.float32)
    b_ih = np.asarray(inputs["b_ih"], np.float32)
    b_hh = np.asarray(inputs["b_hh"], np.float32)
    nat_w = np.asarray(inputs["nat_w"], np.float32)
    morph_w = np.asarray(inputs["morph_w"], np.float32)
    morph_src = np.asarray(inputs["morph_src"], np.int64)
    morph_tgt = np.asarray(inputs["morph_tgt"], np.int64)
    step = int(np.asarray(inputs["step"]))

    np_dt = _np_data_dt()

    # fold the (cell-independent) x part of layer 1 into the bias
    a1_eff = (ba1.astype(np.float64)
              + x[0].astype(np.float64) @ Wa1[:, :IN_DIM].T.astype(np.float64)
              ).astype(np.float32)
    g1_eff = (bg1.astype(np.float64)
              + x[0].astype(np.float64) @ Wg1[:, :IN_DIM].T.astype(np.float64)
              ).astype(np.float32)
    db = ba2 - bg2

    A1 = Wa1[:, IN_DIM:]  # [MLP_H, HID]
    G1 = Wg1[:, IN_DIM:]
    wA1_h = A1.T.reshape(KC, 128, MLP_H).transpose(1, 0, 2)
    wG1_h = G1.T.reshape(KC, 128, MLP_H).transpose(1, 0, 2)
    wW_h = np.concatenate(
        [wA1_h.reshape(128, KC * MLP_H), wG1_h.reshape(128, KC * MLP_H),
         Wa2.T, -Wg2.T], axis=1).astype(np.float32)          # [128, 768]
    bias_h = np.stack([a1_eff, g1_eff, db, np.zeros(128, np.float32)],
                      axis=1).astype(np.float32)             # [128, 4]

    in_maps = []
    for c in range(N_CORES):
        hs = hiddens[c * SHARD:(c + 1) * SHARD]  # [SHARD, HID]
        hT_h = np.ascontiguousarray(
            hs.T.reshape(KC, 128, SHARD).transpose(1, 0, 2))  # [128,KC,SHARD]
        blob_h = np.concatenate(
            [wW_h, bias_h, hT_h[:, :, :NT].reshape(128, KC * NT)],
            axis=1).astype(np_dt, order="C")                 # [128, 1796]
        hT_rest = np.ascontiguousarray(hT_h[:, :, NT:]).astype(np_dt, order="C")
        in_maps.append({"blob": blob_h, "hT": hT_rest})

    nc = _build_nc()
    profile = bool(int(os.environ.get("CTE_PROFILE", "0")))
    if profile:
        profile = _ensure_ntff_hook()
    bres = run_bass_kernel_spmd(nc, in_maps, list(range(N_CORES)),
                                trace=profile)
    LAST_PROFILE.clear()
    LAST_PROFILE["exec_time_ns"] = bres.exec_time_ns
    LAST_PROFILE["mean_exec_time_ns"] = bres.mean_exec_time_ns
    if bres.instructions_and_trace is not None:
        LAST_PROFILE["trace"] = bres.instructions_and_trace

    V = np.zeros(OUT_DIM, np.float64)
    SQ = 0.0
    E = 0.0
    for r in bres.results:
        out = np.asarray(r["res"], np.float64)
        V += out[:, 0]
        SQ += out[:, 1].sum()
        E += out[0, 2]

    combined_out = (V / E).astype(np.float32)[None, :]
    mean_tension = SQ / (OUT_DIM * N_CELLS)

    cat_tension = _morph_cat_tension(
        x, hiddens, Wa1, ba1, Wa2, ba2, Wg1, bg1, Wg2, bg2,
        W_ih, W_hh, b_ih, b_hh, nat_w, morph_w, morph_src, morph_tgt, step)

    avg_tension = np.float32(mean_tension + 0.1 * cat_tension)
    return combined_out, avg_tension


# revision 39
# speedup vs baseline: 1.2301x; 1.1261x over previous
"""Trainium2 Bass kernel for nn_CategoryTheoryEngine (gnn_message_passing).

reference(...) returns only (combined_out [1,128], avg_tension scalar).

Dead-code analysis of the reference:
  - combined_out = softmax(tension) . out  -> needs per-cell out/tension only
  - avg_tension = mean(tension) + 0.1 * cat_tension
  - cat_tension depends on the GRU'd hidden state at the 64 morph_src/tgt
    rows only (limit/colimit are computed BEFORE the +0.05*diff update).
  - faction sync / debate / hid update only affect the discarded 3rd output.

So the device computes, data-parallel over cells (8 cores x 8192 cells):
  per cell: out = MLP_a(h) - MLP_g(h)  (x-part of layer 1 folded into bias),
  sumsq_c = sum_p out^2, w_c = exp(sumsq_c/128), and reduces
  V = sum_c w_c * out_c  [128],  SQ_p = sum_c out^2[p,c]  [128],  E = sum_c w_c.
Host combines the 8 partial results (combined = V/E) and computes the 64
morph rows' GRU -> limit/colimit -> cat_tension in numpy (0.1% of cells).
"""

import os
import sys
import types
from contextlib import ExitStack

import numpy as np

import concourse.bass as bass
import concourse.tile as tile
from concourse import mybir
from concourse.bass_utils import run_bass_kernel_spmd


def _ensure_ntff_hook():
    """The agent image's antenv lacks axon_hooks; recreate it so
    run_bass_kernel_spmd(trace=True) can NTFF-profile through axon."""
    try:
        from antenv.axon_hooks import get_axon_ntff_profile_hook  # noqa: F401

        return True
    except ImportError:
        pass
    try:
        import antenv

        if "/root/.axon_site" not in sys.path:
            sys.path.insert(0, "/root/.axon_site")
        from trn_agent_boot.trn_boot import _ntff_profile_via_ctypes

        mod = types.ModuleType("antenv.axon_hooks")
        state = {"hook": None}
        mod.set_axon_ntff_profile_hook = lambda h: state.__setitem__("hook", h)
        mod.get_axon_ntff_profile_hook = lambda: state["hook"]
        sys.modules["antenv.axon_hooks"] = mod
        antenv.axon_hooks = mod
        mod.set_axon_ntff_profile_hook(
            _ntff_profile_via_ctypes("/opt/axon/libaxon_pjrt.so"))
        return True
    except Exception as e:  # profiling is best-effort only
        print(f"[kernel] ntff hook install failed: {e}")
        return False

def _patch_tail_drain():
    """The stock kernel-tail emits ONE SP Drain waiting every proc's final
    semaphore tick; with 3 engines + several DMA queues that exceeds the
    Drain instruction's sync-wait slots and walrus refuses to codegen.
    Split the waits across several Drain instructions (<=4 waits each)."""
    if getattr(tile.TileContext, "_cte_split_drain", False):
        return
    from concourse.vector_clock import ScopedClock, VectorClock

    def _drain_and_barrier(self, tick_clock, wait_clock):
        gc = tick_clock.global_clock
        n = len(gc)
        procs = [i for i in range(n) if gc[i] > 0]
        groups = [[p] for p in procs] or [[]]
        for grp in groups:
            vec = [gc[i] if i in grp else 0 for i in range(n)]
            drain_inst = self.nc.sync.drain()
            wait_clock.add_sem_waits(
                drain_inst.ins, ScopedClock({None: VectorClock(vec)}))
        self.nc.all_engine_barrier()
        assert self.sems is not None
        popped = self.nc._tile_sem_poison_stack.pop()
        assert popped is self._sem_poison
        self.nc.clear_and_free_semaphores(
            list(self.sems.allocated().values()))
        self.nc.all_engine_barrier()

    tile.TileContext._drain_and_barrier = _drain_and_barrier
    tile.TileContext._cte_split_drain = True


_patch_tail_drain()

N_CELLS, IN_DIM, HID, OUT_DIM, MLP_H, N_MORPH = 65536, 128, 256, 128, 128, 32
N_CORES = 8
SHARD = N_CELLS // N_CORES  # 8192
NT = 512                    # cells per on-chip tile
NTILES = SHARD // NT        # 16
KC = HID // 128             # 2 contraction chunks for layer 1

F32 = mybir.dt.float32
F32R = mybir.dt.float32r
BF16 = mybir.dt.bfloat16

# 'f32r': fp32 storage, full-rate fp32r matmuls.  'bf16': bf16 storage+matmuls
# (halves DMA).  'f32': plain fp32 matmuls (4x slower, debug only).
MM_MODE = os.environ.get("CTE_MM_MODE", "f32r")

LAST_PROFILE = {}

_nc_cache = {}


def _data_dt():
    if MM_MODE == "bf16":
        return BF16
    if MM_MODE == "f32r":
        return F32R
    return F32


def _np_data_dt():
    if MM_MODE == "bf16":
        import ml_dtypes

        return ml_dtypes.bfloat16
    return np.float32


def _mm(ap):
    return ap


def _build_nc():
    if MM_MODE in _nc_cache:
        return _nc_cache[MM_MODE]

    dt = _data_dt()
    AF = mybir.ActivationFunctionType
    ALU = mybir.AluOpType

    nc = bass.Bass()
    # tiles 1..NTILES-1 of the hiddens shard (tile 0 rides in the const blob)
    hT = nc.declare_dram_parameter("hT", [128, KC, SHARD - NT], dt, False)
    # one const blob -> ONE DMA -> one semaphore for everything the first
    # matmul needs (PE matmul tolerates only a single sync wait):
    # free-dim layout: [0:768] weights (A1T k0,k1 | G1T k0,k1 | Wa2T | -Wg2T),
    # [768:771] bias cols (f32 bits: ba1_eff, bg1_eff, db), [771:772] pad,
    # [772:1796] tile-0 h_t (2 chunks x NT)
    CB_W, CB_B, CB_H0, CB_TOT = 0, 768, 772, 772 + KC * NT
    blob = nc.declare_dram_parameter("blob", [128, CB_TOT], dt, False)
    res = nc.declare_dram_parameter("res", [128, 4], F32, True)

    with tile.TileContext(nc) as tc, ExitStack() as ctx:
        consts = ctx.enter_context(tc.tile_pool(name="consts", bufs=1))
        loads = ctx.enter_context(tc.tile_pool(name="loads", bufs=1))
        work = ctx.enter_context(tc.tile_pool(name="work", bufs=6))
        accs = ctx.enter_context(tc.tile_pool(name="accs", bufs=1))
        wsb = ctx.enter_context(tc.tile_pool(name="wsb", bufs=NTILES))
        outs = ctx.enter_context(tc.tile_pool(name="outs", bufs=NTILES))
        # PSUM bank budget (8 banks): p_l1 x1, tp x1, p2 x2, wb x4.
        # wb needs depth 4 so its slot is provably free at schedule time --
        # its reader is the DVE product whose completion PE never observes,
        # and a PE matmul can carry only one sync wait.
        ps1 = ctx.enter_context(tc.tile_pool(name="ps1", bufs=1, space="PSUM"))
        ps2 = ctx.enter_context(tc.tile_pool(name="ps2", bufs=3, space="PSUM"))
        ps4 = ctx.enter_context(tc.tile_pool(name="ps4", bufs=2, space="PSUM"))

        blob_sb = consts.tile([128, CB_TOT], dt)
        nc.gpsimd.dma_start(out=blob_sb[:], in_=blob[:])
        wA1_sb = [blob_sb[:, k * MLP_H:(k + 1) * MLP_H] for k in range(KC)]
        wG1_sb = [blob_sb[:, (KC + k) * MLP_H:(KC + k + 1) * MLP_H]
                  for k in range(KC)]
        wA2_sb = blob_sb[:, 4 * MLP_H:5 * MLP_H]
        wG2n_sb = blob_sb[:, 5 * MLP_H:6 * MLP_H]
        bias_sb = blob_sb[:, CB_B:CB_B + 3].bitcast(F32)
        h0_sb = [blob_sb[:, CB_H0 + k * NT:CB_H0 + (k + 1) * NT]
                 for k in range(KC)]

        # ones produced on ACT (DVE memset cannot write float32r, and ACT
        # keeps the consumers' deps ACT-local): copy(x*0 + 1) = 1
        ones_col = consts.tile([128, 1], dt)
        nc.scalar.activation(out=ones_col[:], in_=bias_sb[:, 0:1],
                             func=AF.Copy, scale=0.0, bias=1.0)
        ones_row = consts.tile([1, 128], dt)
        nc.scalar.activation(out=ones_row[:], in_=blob_sb[0:1, 0:MLP_H],
                             func=AF.Copy, scale=0.0, bias=1.0)
        # zero produced on ACT (not DVE) so the exp's bias dep stays ACT-local;
        # doubles as the ACT pre-touch of the const-blob DMA
        zero1 = consts.tile([1, 1], F32)
        nc.scalar.mul(zero1[:], bias_sb[0:1, 0:1], 0.0)

        # scratch target for DVE guard copies (absorbs PE waits so the
        # product TensorTensor carries only its ACT wait)
        gwp = ctx.enter_context(tc.tile_pool(name="gwp", bufs=8))
        vacc = accs.tile([128, NTILES], F32)
        sqacc = accs.tile([128, NTILES], F32)
        eacc = accs.tile([1, NTILES], F32)

        # the 15 remaining tiles arrive in a few chunked DMAs (pipelining
        # vs. startup bubble; consumers slice the big resident tiles)
        NCHUNK = 5
        per = 3 * NT  # 3 tiles per chunk
        h_chunks = []
        for h in range(NCHUNK):
            lo = h * per
            hi = min(SHARD - NT, lo + per)
            ht_big = loads.tile([128, KC, per], dt, tag=f"ht_big{h}")
            nc.gpsimd.dma_start(out=ht_big[:, :, :hi - lo], in_=hT[:, :, lo:hi])
            h_chunks.append(ht_big)

        prev_prod = [None]

        def emit_mlp(i):
            if i == 0:
                h_k = h0_sb
            else:
                h, off = (i - 1) // 3, ((i - 1) % 3) * NT
                h_k = [h_chunks[h][:, k, off:off + NT] for k in range(KC)]

            # layer 1: a and g sequentially through one single-bank tag
            p1a = ps2.tile([128, NT], F32, tag="p_l1")
            if i > 0 and (i - 1) % 3 == 0:
                # new input chunk: absorb its DMA wait on a standalone
                # ldweights so the first matmul keeps a single sync wait
                nc.tensor.ldweights(h_k[0][:, 0:32].bitcast(BF16))
            for k in range(KC):
                nc.tensor.matmul(
                    p1a[:], _mm(wA1_sb[k]), _mm(h_k[k]),
                    start=(k == 0), stop=(k == KC - 1),
                )
            acts_a = work.tile([128, NT], dt)
            nc.scalar.activation(
                out=acts_a[:], in_=p1a[:], func=AF.Relu,
                bias=bias_sb[:, 0:1], scale=1.0,
            )
            p1g = ps2.tile([128, NT], F32, tag="p_l1")
            for k in range(KC):
                nc.tensor.matmul(
                    p1g[:], _mm(wG1_sb[k]), _mm(h_k[k]),
                    start=(k == 0), stop=(k == KC - 1),
                )
            acts_g = work.tile([128, NT], dt)
            nc.scalar.activation(
                out=acts_g[:], in_=p1g[:], func=AF.Relu,
                bias=bias_sb[:, 1:2], scale=1.0,
            )

            # layer 2: p2 = a' @ Wa2.T - g' @ Wg2.T  (= out - db)
            p2 = ps4.tile([128, NT], F32, tag="p2")
            nc.tensor.matmul(p2[:], _mm(wA2_sb), _mm(acts_a[:]),
                             start=True, stop=False)
            nc.tensor.matmul(p2[:], _mm(wG2n_sb), _mm(acts_g[:]),
                             start=False, stop=True)

            # out = p2 + db, materialized in SBUF (engines may read at most
            # one PSUM operand per instruction)
            out_sb = outs.tile([128, NT], F32)
            nc.scalar.activation(
                out=out_sb[:], in_=p2[:], func=AF.Identity,
                bias=bias_sb[:, 2:3], scale=1.0,
            )

            # sq = out^2 ; ACT accumulator gives sum over cells per partition
            sq = work.tile([128, NT], dt)
            nc.scalar.activation(
                out=sq[:], in_=out_sb[:], func=AF.Square,
                accum_out=sqacc[:, i : i + 1],
            )
            return out_sb, sq

        def emit_tail(i, out_sb, sq):
            # tension*128 per cell: sum over partitions via ones-matmul
            tp = ps1.tile([1, NT], F32)
            nc.tensor.matmul(tp[:], _mm(ones_col[:]), _mm(sq[:]),
                             start=True, stop=True)

            # w = exp(t) ; accumulate sum of w
            w_sb = wsb.tile([1, NT], dt)
            nc.scalar.activation(
                out=w_sb[:], in_=tp[:], func=AF.Exp,
                bias=zero1[0:1, 0:1], scale=1.0 / OUT_DIM,
                accum_out=eacc[:, i : i + 1],
            )

            # broadcast w across partitions (rank-1 matmul). The ldweights
            # guard reads the previous tile's DVE product so PE observes the
            # DVE tick that releases this wb slot.
            wb = ps4.tile([128, NT], F32)
            if prev_prod[0] is not None:
                nc.tensor.ldweights(prev_prod[0][:, 0:32].bitcast(BF16))
            nc.tensor.matmul(wb[:], _mm(ones_row[:]), _mm(w_sb[:]),
                             start=True, stop=True)

            # V partial: sum_c w_c * out[:, c]
            prod = work.tile([128, NT], F32)
            dve_gw = gwp.tile([1, 1], F32, tag="gw1")
            nc.vector.tensor_copy(out=dve_gw[0:1, 0:1], in_=wb[0:1, 0:1])
            dve_gw2 = gwp.tile([1, 1], F32, tag="gw2")
            nc.vector.tensor_copy(out=dve_gw2[0:1, 0:1], in_=out_sb[0:1, 0:1])
            nc.vector.tensor_mul(out=prod[:], in0=out_sb[:], in1=wb[:])
            nc.vector.tensor_reduce(
                out=vacc[:, i : i + 1], in_=prod[:],
                axis=mybir.AxisListType.X, op=ALU.add,
            )
            prev_prod[0] = prod

        # software pipeline: emit tile i's tension tail after tile i+1's
        # MLP matmuls so PE has dense work while ACT computes the exp
        pend = None
        for i in range(NTILES):
            handles = emit_mlp(i)
            if pend is not None:
                emit_tail(i - 1, *pend)
            pend = handles
        emit_tail(NTILES - 1, *pend)

        outsb = consts.tile([128, 4], F32)
        nc.vector.memset(outsb[:], 0.0)
        nc.vector.tensor_reduce(out=outsb[:, 0:1], in_=vacc[:],
                                axis=mybir.AxisListType.X, op=mybir.AluOpType.add)
        nc.vector.tensor_reduce(out=outsb[:, 1:2], in_=sqacc[:],
                                axis=mybir.AxisListType.X, op=mybir.AluOpType.add)
        nc.vector.tensor_reduce(out=outsb[0:1, 2:3], in_=eacc[:],
                                axis=mybir.AxisListType.X, op=mybir.AluOpType.add)
        nc.sync.dma_start(out=res[:], in_=outsb[:])

    nc.finalize()
    _nc_cache[MM_MODE] = nc
    return nc


def _sigmoid(v):
    return 1.0 / (1.0 + np.exp(-v))


def _morph_cat_tension(x, hiddens, Wa1, ba1, Wa2, ba2, Wg1, bg1, Wg2, bg2,
                       W_ih, W_hh, b_ih, b_hh, nat_w, morph_w,
                       morph_src, morph_tgt, step):
    """cat_tension from the 64 morph rows, exact reference math in numpy."""
    needed, inv = np.unique(np.concatenate([morph_src, morph_tgt]),
                            return_inverse=True)
    h_rows = hiddens[needed]  # [R, HID]
    r_n = len(needed)
    comb = np.concatenate(
        [np.broadcast_to(x, (r_n, IN_DIM)), h_rows], axis=1)
    a = np.maximum(comb @ Wa1.T + ba1, 0.0) @ Wa2.T + ba2
    g = np.maximum(comb @ Wg1.T + bg1, 0.0) @ Wg2.T + bg2
    out_r = a - g
    t_r = np.mean(out_r * out_r, axis=-1, keepdims=True)
    mem = np.concatenate([out_r, t_r], axis=1)
    gi = mem @ W_ih.T + b_ih
    gh = h_rows @ W_hh.T + b_hh
    r = _sigmoid(gi[:, :HID] + gh[:, :HID])
    z = _sigmoid(gi[:, HID:2 * HID] + gh[:, HID:2 * HID])
    n_ = np.tanh(gi[:, 2 * HID:] + r * gh[:, 2 * HID:])
    hid_r = (1.0 - z) * n_ + z * h_rows
    if step % 3 == 0:
        hid_r = hid_r @ nat_w.T
    h_src = hid_r[inv[:N_MORPH]]
    h_tgt = hid_r[inv[N_MORPH:]]
    limit = np.einsum("mij,mj->i", morph_w, h_src) / N_MORPH
    colimit = np.einsum("mji,mj->i", morph_w, h_tgt) / N_MORPH
    diff = limit - colimit
    return float(np.mean(diff * diff))


def kernel(**inputs):
    x = np.asarray(inputs["x"], np.float32)
    hiddens = np.asarray(inputs["hiddens"], np.float32)
    Wa1 = np.asarray(inputs["Wa1"], np.float32)
    ba1 = np.asarray(inputs["ba1"], np.float32)
    Wa2 = np.asarray(inputs["Wa2"], np.float32)
    ba2 = np.asarray(inputs["ba2"], np.float32)
    Wg1 = np.asarray(inputs["Wg1"], np.float32)
    bg1 = np.asarray(inputs["bg1"], np.float32)
    Wg2 = np.asarray(inputs["Wg2"], np.float32)
    bg2 = np.asarray(inputs["bg2"], np.float32)
    W_ih = np.asarray(inputs["W_ih"], np.float32)
    W_hh = np.asarray(inputs["W_hh"], np.float32)
    b_ih = np.asarray(inputs["b_ih"], np.float32)
    b_hh = np.asarray(inputs["b_hh"], np.float32)
    nat_w = np.asarray(inputs["nat_w"], np.float32)
    morph_w = np.asarray(inputs["morph_w"], np.float32)
    morph_src = np.asarray(inputs["morph_src"], np.int64)
    morph_tgt = np.asarray(inputs["morph_tgt"], np.int64)
    step = int(np.asarray(inputs["step"]))

    np_dt = _np_data_dt()

    # fold the (cell-independent) x part of layer 1 into the bias
    a1_eff = (ba1.astype(np.float64)
              + x[0].astype(np.float64) @ Wa1[:, :IN_DIM].T.astype(np.float64)
              ).astype(np.float32)
    g1_eff = (bg1.astype(np.float64)
              + x[0].astype(np.float64) @ Wg1[:, :IN_DIM].T.astype(np.float64)
              ).astype(np.float32)
    db = ba2 - bg2

    A1 = Wa1[:, IN_DIM:]  # [MLP_H, HID]
    G1 = Wg1[:, IN_DIM:]
    wA1_h = A1.T.reshape(KC, 128, MLP_H).transpose(1, 0, 2)
    wG1_h = G1.T.reshape(KC, 128, MLP_H).transpose(1, 0, 2)
    wW_h = np.concatenate(
        [wA1_h.reshape(128, KC * MLP_H), wG1_h.reshape(128, KC * MLP_H),
         Wa2.T, -Wg2.T], axis=1).astype(np.float32)          # [128, 768]
    bias_h = np.stack([a1_eff, g1_eff, db, np.zeros(128, np.float32)],
                      axis=1).astype(np.float32)             # [128, 4]

    in_maps = []
    for c in range(N_CORES):
        hs = hiddens[c * SHARD:(c + 1) * SHARD]  # [SHARD, HID]
        hT_h = np.ascontiguousarray(
            hs.T.reshape(KC, 128, SHARD).transpose(1, 0, 2))  # [128,KC,SHARD]
        blob_h = np.concatenate(
            [wW_h, bias_h, hT_h[:, :, :NT].reshape(128, KC * NT)],
            axis=1).astype(np_dt, order="C")                 # [128, 1796]
        hT_rest = np.ascontiguousarray(hT_h[:, :, NT:]).astype(np_dt, order="C")
        in_maps.append({"blob": blob_h, "hT": hT_rest})

    nc = _build_nc()
    profile = bool(int(os.environ.get("CTE_PROFILE", "0")))
    if profile:
        profile = _ensure_ntff_hook()
    bres = run_bass_kernel_spmd(nc, in_maps, list(range(N_CORES)),
                                trace=profile)
    LAST_PROFILE.clear()
    LAST_PROFILE["exec_time_ns"] = bres.exec_time_ns
    LAST_PROFILE["mean_exec_time_ns"] = bres.mean_exec_time_ns
    if bres.instructions_and_trace is not None:
        LAST_PROFILE["trace"] = bres.instructions_and_trace

    V = np.zeros(OUT_DIM, np.float64)
    SQ = 0.0
    E = 0.0
    for r in bres.results:
        out = np.asarray(r["res"], np.float64)
        V += out[:, 0]
        SQ += out[:, 1].sum()
        E += out[0, 2]

    combined_out = (V / E).astype(np.float32)[None, :]
    mean_tension = SQ / (OUT_DIM * N_CELLS)

    cat_tension = _morph_cat_tension(
        x, hiddens, Wa1, ba1, Wa2, ba2, Wg1, bg1, Wg2, bg2,
        W_ih, W_hh, b_ih, b_hh, nat_w, morph_w, morph_src, morph_tgt, step)

    avg_tension = np.float32(mean_tension + 0.1 * cat_tension)
    return combined_out, avg_tension


# revision 40
# speedup vs baseline: 1.3090x; 1.0641x over previous
"""Trainium2 Bass kernel for nn_CategoryTheoryEngine (gnn_message_passing).

reference(...) returns only (combined_out [1,128], avg_tension scalar).

Dead-code analysis of the reference:
  - combined_out = softmax(tension) . out  -> needs per-cell out/tension only
  - avg_tension = mean(tension) + 0.1 * cat_tension
  - cat_tension depends on the GRU'd hidden state at the 64 morph_src/tgt
    rows only (limit/colimit are computed BEFORE the +0.05*diff update).
  - faction sync / debate / hid update only affect the discarded 3rd output.

So the device computes, data-parallel over cells (8 cores x 8192 cells):
  per cell: out = MLP_a(h) - MLP_g(h)  (x-part of layer 1 folded into bias),
  sumsq_c = sum_p out^2, w_c = exp(sumsq_c/128), and reduces
  V = sum_c w_c * out_c  [128],  SQ_p = sum_c out^2[p,c]  [128],  E = sum_c w_c.
Host combines the 8 partial results (combined = V/E) and computes the 64
morph rows' GRU -> limit/colimit -> cat_tension in numpy (0.1% of cells).
"""

import os
import sys
import types
from contextlib import ExitStack

import numpy as np

import concourse.bass as bass
import concourse.tile as tile
from concourse import mybir
from concourse.bass_utils import run_bass_kernel_spmd


def _ensure_ntff_hook():
    """The agent image's antenv lacks axon_hooks; recreate it so
    run_bass_kernel_spmd(trace=True) can NTFF-profile through axon."""
    try:
        from antenv.axon_hooks import get_axon_ntff_profile_hook  # noqa: F401

        return True
    except ImportError:
        pass
    try:
        import antenv

        if "/root/.axon_site" not in sys.path:
            sys.path.insert(0, "/root/.axon_site")
        from trn_agent_boot.trn_boot import _ntff_profile_via_ctypes

        mod = types.ModuleType("antenv.axon_hooks")
        state = {"hook": None}
        mod.set_axon_ntff_profile_hook = lambda h: state.__setitem__("hook", h)
        mod.get_axon_ntff_profile_hook = lambda: state["hook"]
        sys.modules["antenv.axon_hooks"] = mod
        antenv.axon_hooks = mod
        mod.set_axon_ntff_profile_hook(
            _ntff_profile_via_ctypes("/opt/axon/libaxon_pjrt.so"))
        return True
    except Exception as e:  # profiling is best-effort only
        print(f"[kernel] ntff hook install failed: {e}")
        return False

def _patch_tail_drain():
    """The stock kernel-tail emits ONE SP Drain waiting every proc's final
    semaphore tick; with 3 engines + several DMA queues that exceeds the
    Drain instruction's sync-wait slots and walrus refuses to codegen.
    Split the waits across several Drain instructions (<=4 waits each)."""
    if getattr(tile.TileContext, "_cte_split_drain", False):
        return
    from concourse.vector_clock import ScopedClock, VectorClock

    def _drain_and_barrier(self, tick_clock, wait_clock):
        gc = tick_clock.global_clock
        n = len(gc)
        procs = [i for i in range(n) if gc[i] > 0]
        groups = [[p] for p in procs] or [[]]
        for grp in groups:
            vec = [gc[i] if i in grp else 0 for i in range(n)]
            drain_inst = self.nc.sync.drain()
            wait_clock.add_sem_waits(
                drain_inst.ins, ScopedClock({None: VectorClock(vec)}))
        self.nc.all_engine_barrier()
        assert self.sems is not None
        popped = self.nc._tile_sem_poison_stack.pop()
        assert popped is self._sem_poison
        self.nc.clear_and_free_semaphores(
            list(self.sems.allocated().values()))
        self.nc.all_engine_barrier()

    tile.TileContext._drain_and_barrier = _drain_and_barrier
    tile.TileContext._cte_split_drain = True


_patch_tail_drain()

N_CELLS, IN_DIM, HID, OUT_DIM, MLP_H, N_MORPH = 65536, 128, 256, 128, 128, 32
N_CORES = 8
SHARD = N_CELLS // N_CORES  # 8192
NT = 512                    # cells per on-chip tile
NTILES = SHARD // NT        # 16
KC = HID // 128             # 2 contraction chunks for layer 1

F32 = mybir.dt.float32
F32R = mybir.dt.float32r
BF16 = mybir.dt.bfloat16

# 'f32r': fp32 storage, full-rate fp32r matmuls.  'bf16': bf16 storage+matmuls
# (halves DMA).  'f32': plain fp32 matmuls (4x slower, debug only).
MM_MODE = os.environ.get("CTE_MM_MODE", "f32r")

LAST_PROFILE = {}

_nc_cache = {}


def _data_dt():
    if MM_MODE == "bf16":
        return BF16
    if MM_MODE == "f32r":
        return F32R
    return F32


def _np_data_dt():
    if MM_MODE == "bf16":
        import ml_dtypes

        return ml_dtypes.bfloat16
    return np.float32


def _mm(ap):
    return ap


def _build_nc():
    if MM_MODE in _nc_cache:
        return _nc_cache[MM_MODE]

    dt = _data_dt()
    AF = mybir.ActivationFunctionType
    ALU = mybir.AluOpType

    nc = bass.Bass()
    # tiles 1..NTILES-1 of the hiddens shard (tile 0 rides in the const blob)
    hT = nc.declare_dram_parameter("hT", [128, KC, SHARD - NT], dt, False)
    # one const blob -> ONE DMA -> one semaphore for everything the first
    # matmul needs (PE matmul tolerates only a single sync wait):
    # free-dim layout: [0:768] weights (A1T k0,k1 | G1T k0,k1 | Wa2T | -Wg2T),
    # [768:771] bias cols (f32 bits: ba1_eff, bg1_eff, db), [771:772] pad,
    # [772:1796] tile-0 h_t (2 chunks x NT)
    CB_W, CB_B, CB_H0, CB_TOT = 0, 768, 772, 772 + KC * NT
    blob = nc.declare_dram_parameter("blob", [128, CB_TOT], dt, False)
    res = nc.declare_dram_parameter("res", [128, 4], F32, True)

    with tile.TileContext(nc) as tc, ExitStack() as ctx:
        consts = ctx.enter_context(tc.tile_pool(name="consts", bufs=1))
        loads = ctx.enter_context(tc.tile_pool(name="loads", bufs=1))
        work = ctx.enter_context(tc.tile_pool(name="work", bufs=6))
        accs = ctx.enter_context(tc.tile_pool(name="accs", bufs=1))
        wsb = ctx.enter_context(tc.tile_pool(name="wsb", bufs=NTILES))
        outs = ctx.enter_context(tc.tile_pool(name="outs", bufs=NTILES))
        # PSUM bank budget (8 banks): p_l1 x1, tp x1, p2 x2, wb x4.
        # wb needs depth 4 so its slot is provably free at schedule time --
        # its reader is the DVE product whose completion PE never observes,
        # and a PE matmul can carry only one sync wait.
        ps1 = ctx.enter_context(tc.tile_pool(name="ps1", bufs=1, space="PSUM"))
        ps2 = ctx.enter_context(tc.tile_pool(name="ps2", bufs=3, space="PSUM"))
        ps4 = ctx.enter_context(tc.tile_pool(name="ps4", bufs=2, space="PSUM"))

        blob_sb = consts.tile([128, CB_TOT], dt)
        nc.gpsimd.dma_start(out=blob_sb[:], in_=blob[:])
        wA1_sb = [blob_sb[:, k * MLP_H:(k + 1) * MLP_H] for k in range(KC)]
        wG1_sb = [blob_sb[:, (KC + k) * MLP_H:(KC + k + 1) * MLP_H]
                  for k in range(KC)]
        wA2_sb = blob_sb[:, 4 * MLP_H:5 * MLP_H]
        wG2n_sb = blob_sb[:, 5 * MLP_H:6 * MLP_H]
        bias_sb = blob_sb[:, CB_B:CB_B + 3].bitcast(F32)
        h0_sb = [blob_sb[:, CB_H0 + k * NT:CB_H0 + (k + 1) * NT]
                 for k in range(KC)]

        # ones produced on ACT (DVE memset cannot write float32r, and ACT
        # keeps the consumers' deps ACT-local): copy(x*0 + 1) = 1
        ones_col = consts.tile([128, 1], dt)
        nc.scalar.activation(out=ones_col[:], in_=bias_sb[:, 0:1],
                             func=AF.Copy, scale=0.0, bias=1.0)
        ones_row = consts.tile([1, 128], dt)
        nc.scalar.activation(out=ones_row[:], in_=blob_sb[0:1, 0:MLP_H],
                             func=AF.Copy, scale=0.0, bias=1.0)
        # zero produced on ACT (not DVE) so the exp's bias dep stays ACT-local;
        # doubles as the ACT pre-touch of the const-blob DMA
        zero1 = consts.tile([1, 1], F32)
        nc.scalar.mul(zero1[:], bias_sb[0:1, 0:1], 0.0)

        # scratch target for DVE guard copies (absorbs PE waits so the
        # product TensorTensor carries only its ACT wait)
        gwp = ctx.enter_context(tc.tile_pool(name="gwp", bufs=8))
        vacc = accs.tile([128, NTILES], F32)
        sqacc = accs.tile([128, NTILES], F32)
        eacc = accs.tile([1, NTILES], F32)

        # the 15 remaining tiles arrive in a few chunked DMAs (pipelining
        # vs. startup bubble; consumers slice the big resident tiles)
        NCHUNK = 5
        per = 3 * NT  # 3 tiles per chunk
        h_chunks = []
        for h in range(NCHUNK):
            lo = h * per
            hi = min(SHARD - NT, lo + per)
            ht_big = loads.tile([128, KC, per], dt, tag=f"ht_big{h}")
            nc.gpsimd.dma_start(out=ht_big[:, :, :hi - lo], in_=hT[:, :, lo:hi])
            h_chunks.append(ht_big)

        prev_prod = [None]

        def emit_mlp(i):
            if i == 0:
                h_k = h0_sb
            else:
                h, off = (i - 1) // 3, ((i - 1) % 3) * NT
                h_k = [h_chunks[h][:, k, off:off + NT] for k in range(KC)]

            # layer 1: a and g sequentially through one single-bank tag
            p1a = ps2.tile([128, NT], F32, tag="p_l1")
            if i > 0 and (i - 1) % 3 == 0:
                # new input chunk: absorb its DMA wait on a standalone
                # ldweights so the first matmul keeps a single sync wait
                nc.tensor.ldweights(h_k[0][:, 0:32].bitcast(BF16))
            for k in range(KC):
                nc.tensor.matmul(
                    p1a[:], _mm(wA1_sb[k]), _mm(h_k[k]),
                    start=(k == 0), stop=(k == KC - 1),
                )
            acts_a = work.tile([128, NT], dt)
            nc.scalar.activation(
                out=acts_a[:], in_=p1a[:], func=AF.Relu,
                bias=bias_sb[:, 0:1], scale=1.0,
            )
            p1g = ps2.tile([128, NT], F32, tag="p_l1")
            for k in range(KC):
                nc.tensor.matmul(
                    p1g[:], _mm(wG1_sb[k]), _mm(h_k[k]),
                    start=(k == 0), stop=(k == KC - 1),
                )
            acts_g = work.tile([128, NT], dt)
            nc.scalar.activation(
                out=acts_g[:], in_=p1g[:], func=AF.Relu,
                bias=bias_sb[:, 1:2], scale=1.0,
            )

            # layer 2: p2 = a' @ Wa2.T - g' @ Wg2.T  (= out - db)
            p2 = ps4.tile([128, NT], F32, tag="p2")
            nc.tensor.matmul(p2[:], _mm(wA2_sb), _mm(acts_a[:]),
                             start=True, stop=False)
            nc.tensor.matmul(p2[:], _mm(wG2n_sb), _mm(acts_g[:]),
                             start=False, stop=True)

            # out = p2 + db, materialized in SBUF (engines may read at most
            # one PSUM operand per instruction)
            out_sb = outs.tile([128, NT], F32)
            nc.scalar.activation(
                out=out_sb[:], in_=p2[:], func=AF.Identity,
                bias=bias_sb[:, 2:3], scale=1.0,
            )

            # sq = out^2 ; ACT accumulator gives sum over cells per partition
            sq = work.tile([128, NT], dt)
            nc.scalar.activation(
                out=sq[:], in_=out_sb[:], func=AF.Square,
                accum_out=sqacc[:, i : i + 1],
            )
            return out_sb, sq

        def emit_tail(i, out_sb, sq):
            # tension*128 per cell: sum over partitions via ones-matmul
            tp = ps1.tile([1, NT], F32)
            nc.tensor.matmul(tp[:], _mm(ones_col[:]), _mm(sq[:]),
                             start=True, stop=True)

            # w = exp(t) ; accumulate sum of w
            w_sb = wsb.tile([1, NT], dt)
            nc.scalar.activation(
                out=w_sb[:], in_=tp[:], func=AF.Exp,
                bias=zero1[0:1, 0:1], scale=1.0 / OUT_DIM,
                accum_out=eacc[:, i : i + 1],
            )

            # broadcast w across partitions (rank-1 matmul). The ldweights
            # guard reads the previous tile's DVE product so PE observes the
            # DVE tick that releases this wb slot.
            wb = ps4.tile([128, NT], F32)
            if prev_prod[0] is not None:
                nc.tensor.ldweights(prev_prod[0][:, 0:32].bitcast(BF16))
            nc.tensor.matmul(wb[:], _mm(ones_row[:]), _mm(w_sb[:]),
                             start=True, stop=True)

            # V partial: sum_c w_c * out[:, c]
            prod = work.tile([128, NT], F32)
            dve_gw = gwp.tile([1, 1], F32, tag="gw1")
            nc.vector.tensor_copy(out=dve_gw[0:1, 0:1], in_=wb[0:1, 0:1])
            dve_gw2 = gwp.tile([1, 1], F32, tag="gw2")
            nc.vector.tensor_copy(out=dve_gw2[0:1, 0:1], in_=out_sb[0:1, 0:1])
            nc.vector.tensor_mul(out=prod[:], in0=out_sb[:], in1=wb[:])
            nc.vector.tensor_reduce(
                out=vacc[:, i : i + 1], in_=prod[:],
                axis=mybir.AxisListType.X, op=ALU.add,
            )
            prev_prod[0] = prod

        # software pipeline: emit tile i's tension tail two tiles behind the
        # MLP so PE has dense matmul work while ACT computes the exp
        pend = []
        for i in range(NTILES):
            pend.append(emit_mlp(i))
            if len(pend) > 2:
                emit_tail(i - 2, *pend.pop(0))
        for j, h in enumerate(pend):
            emit_tail(NTILES - len(pend) + j, *h)

        outsb = consts.tile([128, 4], F32)
        nc.vector.memset(outsb[:], 0.0)
        nc.vector.tensor_reduce(out=outsb[:, 0:1], in_=vacc[:],
                                axis=mybir.AxisListType.X, op=mybir.AluOpType.add)
        nc.vector.tensor_reduce(out=outsb[:, 1:2], in_=sqacc[:],
                                axis=mybir.AxisListType.X, op=mybir.AluOpType.add)
        nc.vector.tensor_reduce(out=outsb[0:1, 2:3], in_=eacc[:],
                                axis=mybir.AxisListType.X, op=mybir.AluOpType.add)
        nc.sync.dma_start(out=res[:], in_=outsb[:])

    nc.finalize()
    _nc_cache[MM_MODE] = nc
    return nc


def _sigmoid(v):
    return 1.0 / (1.0 + np.exp(-v))


def _morph_cat_tension(x, hiddens, Wa1, ba1, Wa2, ba2, Wg1, bg1, Wg2, bg2,
                       W_ih, W_hh, b_ih, b_hh, nat_w, morph_w,
                       morph_src, morph_tgt, step):
    """cat_tension from the 64 morph rows, exact reference math in numpy."""
    needed, inv = np.unique(np.concatenate([morph_src, morph_tgt]),
                            return_inverse=True)
    h_rows = hiddens[needed]  # [R, HID]
    r_n = len(needed)
    comb = np.concatenate(
        [np.broadcast_to(x, (r_n, IN_DIM)), h_rows], axis=1)
    a = np.maximum(comb @ Wa1.T + ba1, 0.0) @ Wa2.T + ba2
    g = np.maximum(comb @ Wg1.T + bg1, 0.0) @ Wg2.T + bg2
    out_r = a - g
    t_r = np.mean(out_r * out_r, axis=-1, keepdims=True)
    mem = np.concatenate([out_r, t_r], axis=1)
    gi = mem @ W_ih.T + b_ih
    gh = h_rows @ W_hh.T + b_hh
    r = _sigmoid(gi[:, :HID] + gh[:, :HID])
    z = _sigmoid(gi[:, HID:2 * HID] + gh[:, HID:2 * HID])
    n_ = np.tanh(gi[:, 2 * HID:] + r * gh[:, 2 * HID:])
    hid_r = (1.0 - z) * n_ + z * h_rows
    if step % 3 == 0:
        hid_r = hid_r @ nat_w.T
    h_src = hid_r[inv[:N_MORPH]]
    h_tgt = hid_r[inv[N_MORPH:]]
    limit = np.einsum("mij,mj->i", morph_w, h_src) / N_MORPH
    colimit = np.einsum("mji,mj->i", morph_w, h_tgt) / N_MORPH
    diff = limit - colimit
    return float(np.mean(diff * diff))


def kernel(**inputs):
    x = np.asarray(inputs["x"], np.float32)
    hiddens = np.asarray(inputs["hiddens"], np.float32)
    Wa1 = np.asarray(inputs["Wa1"], np.float32)
    ba1 = np.asarray(inputs["ba1"], np.float32)
    Wa2 = np.asarray(inputs["Wa2"], np.float32)
    ba2 = np.asarray(inputs["ba2"], np.float32)
    Wg1 = np.asarray(inputs["Wg1"], np.float32)
    bg1 = np.asarray(inputs["bg1"], np.float32)
    Wg2 = np.asarray(inputs["Wg2"], np.float32)
    bg2 = np.asarray(inputs["bg2"], np.float32)
    W_ih = np.asarray(inputs["W_ih"], np.float32)
    W_hh = np.asarray(inputs["W_hh"], np.float32)
    b_ih = np.asarray(inputs["b_ih"], np.float32)
    b_hh = np.asarray(inputs["b_hh"], np.float32)
    nat_w = np.asarray(inputs["nat_w"], np.float32)
    morph_w = np.asarray(inputs["morph_w"], np.float32)
    morph_src = np.asarray(inputs["morph_src"], np.int64)
    morph_tgt = np.asarray(inputs["morph_tgt"], np.int64)
    step = int(np.asarray(inputs["step"]))

    np_dt = _np_data_dt()

    # fold the (cell-independent) x part of layer 1 into the bias
    a1_eff = (ba1.astype(np.float64)
              + x[0].astype(np.float64) @ Wa1[:, :IN_DIM].T.astype(np.float64)
              ).astype(np.float32)
    g1_eff = (bg1.astype(np.float64)
              + x[0].astype(np.float64) @ Wg1[:, :IN_DIM].T.astype(np.float64)
              ).astype(np.float32)
    db = ba2 - bg2

    A1 = Wa1[:, IN_DIM:]  # [MLP_H, HID]
    G1 = Wg1[:, IN_DIM:]
    wA1_h = A1.T.reshape(KC, 128, MLP_H).transpose(1, 0, 2)
    wG1_h = G1.T.reshape(KC, 128, MLP_H).transpose(1, 0, 2)
    wW_h = np.concatenate(
        [wA1_h.reshape(128, KC * MLP_H), wG1_h.reshape(128, KC * MLP_H),
         Wa2.T, -Wg2.T], axis=1).astype(np.float32)          # [128, 768]
    bias_h = np.stack([a1_eff, g1_eff, db, np.zeros(128, np.float32)],
                      axis=1).astype(np.float32)             # [128, 4]

    in_maps = []
    for c in range(N_CORES):
        hs = hiddens[c * SHARD:(c + 1) * SHARD]  # [SHARD, HID]
        hT_h = np.ascontiguousarray(
            hs.T.reshape(KC, 128, SHARD).transpose(1, 0, 2))  # [128,KC,SHARD]
        blob_h = np.concatenate(
            [wW_h, bias_h, hT_h[:, :, :NT].reshape(128, KC * NT)],
            axis=1).astype(np_dt, order="C")                 # [128, 1796]
        hT_rest = np.ascontiguousarray(hT_h[:, :, NT:]).astype(np_dt, order="C")
        in_maps.append({"blob": blob_h, "hT": hT_rest})

    nc = _build_nc()
    profile = bool(int(os.environ.get("CTE_PROFILE", "0")))
    if profile:
        profile = _ensure_ntff_hook()
    bres = run_bass_kernel_spmd(nc, in_maps, list(range(N_CORES)),
                                trace=profile)
    LAST_PROFILE.clear()
    LAST_PROFILE["exec_time_ns"] = bres.exec_time_ns
    LAST_PROFILE["mean_exec_time_ns"] = bres.mean_exec_time_ns
    if bres.instructions_and_trace is not None:
        LAST_PROFILE["trace"] = bres.instructions_and_trace

    V = np.zeros(OUT_DIM, np.float64)
    SQ = 0.0
    E = 0.0
    for r in bres.results:
        out = np.asarray(r["res"], np.float64)
        V += out[:, 0]
        SQ += out[:, 1].sum()
        E += out[0, 2]

    combined_out = (V / E).astype(np.float32)[None, :]
    mean_tension = SQ / (OUT_DIM * N_CELLS)

    cat_tension = _morph_cat_tension(
        x, hiddens, Wa1, ba1, Wa2, ba2, Wg1, bg1, Wg2, bg2,
        W_ih, W_hh, b_ih, b_hh, nat_w, morph_w, morph_src, morph_tgt, step)

    avg_tension = np.float32(mean_tension + 0.1 * cat_tension)
    return combined_out, avg_tension
